# revision 36
# baseline (speedup 1.0000x reference)
"""Trainium2 Bass kernel for AdvancedTransformerEncoderBlock (fp8 DoubleRow).

Sharding: token-parallel across 8 cores (B=2 x 4 seq chunks of 512), each core
recomputes a 256-token K/V halo -> zero collectives.

Precision plan (validated vs fp32 reference, rel_err ~= 0.015):
  - qkv proj:   fp8e4 DoubleRow, weights split hi+lo(x16), activation split
                hi + hi/16 + residual  (3 passes, 4x per-pass speedup)
  - attention:  bf16 (transposed-logits flow: logits land [keys, queries] in
                PSUM; exp on Act; band mask folded into the PSUM->SBUF copy as
                a 0/1 multiply; softmax sums via ones[128,64] matmul so the
                per-query denominators arrive broadcast across partitions;
                normalize folded into the o2 copy)
  - out proj:   fp8e4 DoubleRow single-pass (o2/wo plain fp8)
  - gate/up:    like qkv (3 passes)
  - down proj:  weights split fp8(4w) + fp8(32*res), H plain fp8 + H/8 copy;
                the 4x weight prescale (keeps wd out of fp8 subnormals) is
                undone by a 0.25 scale folded into the PSUM->SBUF copy
PSUM accumulation stays fp32, residual stream stays fp32.
RoPE rotate-half runs as a PE permutation matmul.
Attention runs one query-tile ahead on logits so exp/mask latency hides under
sums/AV of the previous tile plus the interleaved projection fillers.
"""

import numpy as np

B, S, D, F, H, HD = 2, 2048, 1024, 4096, 16, 64
WIN = 256
NCORES = 8
CH = 4           # chunks per batch
CS = S // CH     # 512 tokens per chunk (queries)
HT = CS + WIN    # 768 tokens incl. halo (keys/values)
NQT = CS // 128  # 4 query tiles
EPS = 1e-5
QKV_THIRD = True   # include activation-residual pass in qkv proj
GU_THIRD = True    # include activation-residual pass in gate/up


def build_program(has_bv=False, has_bg=False, has_bd=False):
    import concourse.bass as bass
    import concourse.bacc as bacc_mod
    import concourse.tile as tile
    import concourse.mybir as mybir
    from concourse.masks import make_identity
    from contextlib import ExitStack

    dt = mybir.dt
    f32, bf16, f8 = dt.float32, dt.bfloat16, dt.float8e4
    AF = mybir.ActivationFunctionType
    OP = mybir.AluOpType
    DR = mybir.MatmulPerfMode.DoubleRow

    nc = bacc_mod.Bacc()
    Pf = lambda name, shape: nc.declare_dram_parameter(name, list(shape), f32, isOutput=False)
    Pb = lambda name, shape: nc.declare_dram_parameter(name, list(shape), bf16, isOutput=False)
    P8 = lambda name, shape: nc.declare_dram_parameter(name, list(shape), f8, isOutput=False)

    xh_d = Pb("xh", (HT, D))
    wqk_d = P8("wqk", (8, 128, 4, 4, 2, 128))   # [mt][p][qhi,qlo,khi,klo][pair][i][m]
    wv_d = P8("wv", (128, 2, 4, 2, 1024))       # [p][hi/lo][pair][i][n]
    wo_d = P8("wo", (128, 4, 2, 1024))          # [p][pair][i][n]
    wgu_d = P8("wgu", (32, 128, 2, 2, 4, 2, 128))  # [mt][p][g/u][hi/lo][pair][i][m]
    wd_d = P8("wd", (16, 128, 2, 2, 1024))      # [pair][p][hi/lo][i][n]
    bv_d = Pb("bv", (1, D))
    bd_d = Pb("bd", (1, D))
    bg_d = Pb("bg", (1, F))
    cbf_d = Pf("cbf", (128, 48))                # bqk [:,0:16], bu [:,16:48]
    cbb_d = Pb("cbb", (128, 3200))
    out_d = nc.declare_dram_parameter("out", [CS, D], f32, isOutput=True)

    with tile.TileContext(nc) as tc, ExitStack() as top:
        const = top.enter_context(tc.tile_pool(name="const", bufs=1))

        # x tiles first: their DMAs head the queue so LN/transposes start early
        x_pool = top.enter_context(tc.tile_pool(name="x", bufs=6))
        x_tiles = []
        for tt in range(6):
            xt = x_pool.tile([128, D], bf16, tag="xt")
            eng = nc.sync if tt % 2 == 0 else nc.gpsimd
            if tt == 0:
                eng.dma_start(out=xt[:, 0:512], in_=xh_d[0:128, 0:512])
                eng.dma_start(out=xt[:, 512:1024], in_=xh_d[0:128, 512:1024])
            else:
                eng.dma_start(out=xt, in_=xh_d[tt * 128:(tt + 1) * 128, :])
            x_tiles.append(xt)

        # ---- constants ----
        cbf = const.tile([128, 48], f32, tag="cbf")
        nc.sync.dma_start(out=cbf, in_=cbf_d[:, :])
        cbb = const.tile([128, 3200], bf16, tag="cbb")
        nc.gpsimd.dma_start(out=cbb, in_=cbb_d[:, :])
        bqk_sb = cbf[:, 0:16]
        bu_sb = cbf[:, 16:48]
        cosk = cbb[:, 0:768]
        msink = cbb[:, 768:1536]
        cosq = cbb[:, WIN:768]
        msinq = cbb[:, 768 + WIN:1536]
        masks = [cbb[:, 1536 + qt * 384:1536 + (qt + 1) * 384] for qt in range(NQT)]
        pshuf = cbb[:, 3072:3200]

        identb = const.tile([128, 128], bf16, tag="identb")
        make_identity(nc, identb)
        ones8d = const.tile([128, 2, 64], f8, tag="ones8d")
        nc.vector.memset(ones8d, 1.0)
        ones_row = const.tile([1, 512], bf16, tag="ones_row")
        nc.vector.memset(ones_row, 1.0)
        eps_t = const.tile([128, 1], f32, tag="eps")
        nc.vector.memset(eps_t, EPS)
        esh_t = const.tile([128, 1], f32, tag="esh")
        nc.vector.memset(esh_t, -3.0)
        if has_bv:
            bv_sb = const.tile([1, D], bf16, tag="bv")
            nc.sync.dma_start(out=bv_sb, in_=bv_d[:, :])
        if has_bd:
            bd_sb = const.tile([1, D], bf16, tag="bd")
            nc.sync.dma_start(out=bd_sb, in_=bd_d[:, :])
        if has_bg:
            bg_sb = const.tile([1, F], bf16, tag="bg")
            nc.sync.dma_start(out=bg_sb, in_=bg_d[:, :])

        # ---- persistent activation pools ----
        x2_pool = top.enter_context(tc.tile_pool(name="x2", bufs=4))
        y2T_pool = top.enter_context(tc.tile_pool(name="y2T", bufs=4))
        o2_pool = top.enter_context(tc.tile_pool(name="o2", bufs=4))

        def ln_stats(src, tmp_pool):
            stats = tmp_pool.tile([128, 2, 6], f32, tag="lnstats")
            mv = tmp_pool.tile([128, 2], f32, tag="lnmv")
            for sg in range(2):
                nc.vector.bn_stats(out=stats[:, sg, :], in_=src[:, sg * 512:(sg + 1) * 512])
            nc.vector.bn_aggr(out=mv, in_=stats)
            return mv

        def ln_norm(src, dst, mv, tmp_pool):
            rs = tmp_pool.tile([128, 1], f32, tag="lnrs")
            nc.scalar.activation(out=rs, in_=mv[:, 1:2], func=AF.Sqrt,
                                 bias=eps_t, scale=1.0)
            nc.vector.reciprocal(out=rs, in_=rs)
            nb = tmp_pool.tile([128, 1], f32, tag="lnnb")
            nc.vector.tensor_scalar(out=nb, in0=mv[:, 0:1], scalar1=rs,
                                    scalar2=-1.0, op0=OP.mult, op1=OP.mult)
            nc.scalar.activation(out=dst, in_=src, func=AF.Identity,
                                 bias=nb, scale=rs)

        def layernorm(src, dst, tmp_pool):
            ln_norm(src, dst, ln_stats(src, tmp_pool), tmp_pool)

        # mid-lived: x2-LN workspace + y2 (read by the late y2T transposes)
        mid_scope = ExitStack()
        ln_tmp2 = mid_scope.enter_context(tc.tile_pool(name="ln_tmp2", bufs=3))
        y2_pool = mid_scope.enter_context(tc.tile_pool(name="y2", bufs=4))

        qkv_scope = ExitStack()
        yT_pool = qkv_scope.enter_context(tc.tile_pool(name="yT", bufs=4))
        wo_pool = qkv_scope.enter_context(tc.tile_pool(name="wo", bufs=1))
        wo8 = wo_pool.tile([128, 4, 2, 1024], f8, tag="wo8")
        qT_pool = qkv_scope.enter_context(tc.tile_pool(name="qT", bufs=3))
        kT_pool = qkv_scope.enter_context(tc.tile_pool(name="kT", bufs=3))
        vb_pool = qkv_scope.enter_context(tc.tile_pool(name="vb", bufs=6))


        # y^T pair tiles split by token halves so consumers start after the
        # first three LN outputs: a = tokens 0:384, b = 384:768
        HH = HT // 2
        yT8a = [yT_pool.tile([128, 2, HH], f8, name="yT8a", tag="yT8a") for _ in range(4)]
        yT8b = [yT_pool.tile([128, 2, HH], f8, name="yT8b", tag="yT8b") for _ in range(4)]
        yT8sa = [yT_pool.tile([128, 2, HH], f8, name="yT8sa", tag="yT8sa") for _ in range(4)]
        yT8sb = [yT_pool.tile([128, 2, HH], f8, name="yT8sb", tag="yT8sb") for _ in range(4)]
        yTr8a = [yT_pool.tile([128, 2, HH], f8, name="yTr8a", tag="yTr8a")
                 for _ in range(4)] if QKV_THIRD else None
        yTr8b = [yT_pool.tile([128, 2, HH], f8, name="yTr8b", tag="yTr8b")
                 for _ in range(4)] if QKV_THIRD else None

        # =========== phase A: LN1 -> y -> y^T fp8 triplet ===========
        with ExitStack() as ph:
            ln_tmp = ph.enter_context(tc.tile_pool(name="ln_tmp", bufs=6))
            y_pool = ph.enter_context(tc.tile_pool(name="y", bufs=6))
            pst = ph.enter_context(tc.tile_pool(name="pst", bufs=8, space="PSUM"))

            ys = []
            for tt in range(6):
                y = y_pool.tile([128, D], bf16, tag="y")
                layernorm(x_tiles[tt], y, ln_tmp)
                ys.append(y)
            # tt-outer with half-granular copies: the a-half (tokens 0:384)
            # ships as soon as the first three LN outputs exist
            pts = [pst.tile([128, 6, 128], bf16, name="pt", tag="pst")
                   for _ in range(8)]
            for half, (hi_l, s_l, r_l) in enumerate(
                    [(yT8a, yT8sa, yTr8a), (yT8b, yT8sb, yTr8b)]):
                for tt in range(half * 3, half * 3 + 3):
                    for dtl in range(8):
                        nc.tensor.transpose(pts[dtl][:, tt, :],
                                            ys[tt][:, dtl * 128:(dtl + 1) * 128],
                                            identb)
                for dtl in range(8):
                    pt = pts[dtl][:, half * 3:half * 3 + 3, :]
                    dst_hi = hi_l[dtl // 2][:, dtl % 2, :]
                    nc.scalar.activation(out=dst_hi, in_=pt, func=AF.Identity)
                    nc.gpsimd.tensor_scalar_mul(out=s_l[dtl // 2][:, dtl % 2, :],
                                                in0=dst_hi, scalar1=1.0 / 16)
                    if QKV_THIRD:
                        nc.vector.tensor_tensor(out=r_l[dtl // 2][:, dtl % 2, :],
                                                in0=pt, in1=dst_hi, op=OP.subtract)

        # late-lived pools opened after phase A so their space reuses the LN
        # workspace; DMAs for v/out-proj weights head the queue here
        wqkp_scope = ExitStack()
        wqk_pool = wqkp_scope.enter_context(tc.tile_pool(name="wqkp", bufs=3))
        mlp_scope = ExitStack()
        hh_pool = mlp_scope.enter_context(tc.tile_pool(name="hh", bufs=16))
        wd_pool = mlp_scope.enter_context(tc.tile_pool(name="wd", bufs=4))
        wgu0_pool = mlp_scope.enter_context(tc.tile_pool(name="wgu0", bufs=2))
        vw_scope = ExitStack()
        wv_pool = vw_scope.enter_context(tc.tile_pool(name="wv", bufs=1))
        wv8 = wv_pool.tile([128, 2, 4, 2, 1024], f8, tag="wv8")
        for p_ in range(4):
            eng = nc.sync if p_ % 2 == 0 else nc.gpsimd
            eng.dma_start(out=wv8[:, :, p_, :, :], in_=wv_d[:, :, p_, :, :])
        nc.gpsimd.dma_start(out=wo8, in_=wo_d[:, :, :, :])

        qkv_terms = [(yT8a, yT8b, 0), (yT8sa, yT8sb, 1)] + \
            ([(yTr8a, yTr8b, 0)] if QKV_THIRD else [])

        def ytok(term, lo, hi):
            """AP for token range [lo, hi) of a qkv term (within one half)."""
            a_l, b_l, _ = term
            if hi <= HH:
                return lambda p: a_l[p][:, :, lo:hi]
            assert lo >= HH
            return lambda p: b_l[p][:, :, lo - HH:hi - HH]

        # =========== phase B: v projection (fp8 DR, pair-outer) ===========
        with ExitStack() as ph:
            psv = ph.enter_context(tc.tile_pool(name="psv", bufs=6, space="PSUM"))
            v8 = vb_pool.tile([128, 6, D], f8, tag="vbf")
            nterm = len(qkv_terms)
            for chv in range(2):
                pv = [psv.tile([128, 512], f32, name="psv", tag="psv") for _ in range(6)]
                for ti, term in enumerate(qkv_terms):
                    hl = term[2]
                    for p in range(4):
                        for tt in range(6):
                            lsrc = ytok(term, tt * 128, (tt + 1) * 128)(p)
                            for cn in range(2):
                                reg = pv[tt][:, cn * 256:(cn + 1) * 256]
                                last = (p == 3 and ti == nterm - 1)
                                nc.tensor.matmul(
                                    reg,
                                    lhsT=lsrc,
                                    rhs=wv8[:, hl, p, :,
                                            chv * 512 + cn * 256:chv * 512 + (cn + 1) * 256],
                                    start=(ti == 0 and p == 0 and cn == 0),
                                    stop=(last and not has_bv), perf_mode=DR)
                if has_bv:
                    for tt in range(6):
                        for cn in range(2):
                            nc.tensor.matmul(pv[tt][:, cn * 256:(cn + 1) * 256],
                                             lhsT=ones_row[:, 0:128],
                                             rhs=bv_sb[:, chv * 512 + cn * 256:
                                                       chv * 512 + (cn + 1) * 256],
                                             start=False, stop=True)
                for tt in range(6):
                    sl = slice(chv * 512, (chv + 1) * 512)
                    if tt % 2 == 0:
                        nc.scalar.copy(out=v8[:, tt, sl], in_=pv[tt])
                    else:
                        nc.vector.tensor_copy(out=v8[:, tt, sl], in_=pv[tt])
        vw_scope.close()

        # ==== phase C: q/k proj + RoPE pipelined with attention (flow B) ====
        qT, kT = [], []
        with ExitStack() as ph:
            psb = ph.enter_context(tc.tile_pool(name="psb", bufs=3, space="PSUM"))
            rope_tmp = ph.enter_context(tc.tile_pool(name="rope_tmp", bufs=2))
            at = ph.enter_context(tc.tile_pool(name="at", bufs=3))
            psl = ph.enter_context(tc.tile_pool(name="psl", bufs=2, space="PSUM"))
            pss = ph.enter_context(tc.tile_pool(name="pss", bufs=1, space="PSUM"))
            pso = ph.enter_context(tc.tile_pool(name="pso", bufs=2, space="PSUM"))

            o28 = [o2_pool.tile([128, 2, CS], f8, name="o28", tag="o28")
                   for _ in range(4)]

            wqk_tiles = {}

            def issue_wqk(mt):
                w = wqk_pool.tile([128, 4, 4, 2, 128], f8, name="wqk", tag="wqk")
                nc.sync.dma_start(out=w, in_=wqk_d[mt])
                wqk_tiles[mt] = w

            def proj_chunks(mt):
                w = wqk_tiles.pop(mt)
                qt_t = qT_pool.tile([128, CS], bf16, tag="qT")
                kt_t = kT_pool.tile([128, HT], bf16, tag="kT")
                st = {}

                def emit_proj(ps, wbase, tok_lo, tok_hi):
                    cuts = sorted({tok_lo, tok_hi}
                                  | {c for c in (HH, 256, 640) if tok_lo < c < tok_hi})
                    chunks = list(zip(cuts[:-1], cuts[1:]))
                    for cn, (c0, c1) in enumerate(chunks):
                        reg = ps[:, c0 - tok_lo:c1 - tok_lo]
                        for ti, term in enumerate(qkv_terms):
                            hl = term[2]
                            for p in range(4):
                                nc.tensor.matmul(
                                    reg,
                                    lhsT=w[:, wbase + hl, p, :, :],
                                    rhs=ytok(term, c0, c1)(p),
                                    start=(ti == 0 and p == 0 and cn == 0),
                                    stop=(ti == len(qkv_terms) - 1 and p == 3),
                                    perf_mode=DR)

                def c0():  # q projection
                    ps = psb.tile([128, CS], f32, tag="psqk")
                    emit_proj(ps, 0, WIN, HT)
                    qb = rope_tmp.tile([128, CS], bf16, tag="ropesrc")
                    nc.scalar.activation(out=qb, in_=ps, func=AF.Identity,
                                         bias=bqk_sb[:, mt:mt + 1], scale=1.0)
                    st["qb"] = qb

                def c1():  # q rope
                    qb = st["qb"]
                    pr = psb.tile([128, 512], f32, tag="psqk")
                    nc.tensor.matmul(pr, lhsT=pshuf, rhs=qb, start=True, stop=True)
                    u = rope_tmp.tile([128, HT], bf16, tag="ropeu")
                    nc.vector.tensor_mul(out=u[:, :CS], in0=qb, in1=cosq)
                    t1 = rope_tmp.tile([128, 512], bf16, tag="ropet")
                    nc.vector.tensor_mul(out=t1, in0=pr, in1=msinq)
                    nc.vector.tensor_add(out=qt_t, in0=u[:, :CS], in1=t1)

                def c2():  # k projection half 0
                    kb = rope_tmp.tile([128, HT], bf16, tag="ropesrck")
                    st["kb"] = kb
                    ps = psb.tile([128, 384], f32, tag="psqk")
                    emit_proj(ps, 2, 0, 384)
                    nc.scalar.activation(out=kb[:, 0:384], in_=ps, func=AF.Identity,
                                         bias=bqk_sb[:, 8 + mt:9 + mt], scale=1.0)

                def c3():  # k projection half 1 + k rope
                    kb = st["kb"]
                    ps = psb.tile([128, 384], f32, tag="psqk")
                    emit_proj(ps, 2, 384, HT)
                    nc.scalar.activation(out=kb[:, 384:768], in_=ps, func=AF.Identity,
                                         bias=bqk_sb[:, 8 + mt:9 + mt], scale=1.0)
                    u = rope_tmp.tile([128, HT], bf16, tag="ropeu")
                    nc.vector.tensor_mul(out=u, in0=kb, in1=cosk)
                    for c in range(2):
                        w_ = 512 if c == 0 else 256
                        sl_ = slice(c * 512, c * 512 + w_)
                        pr = psb.tile([128, 512], f32, tag="psqk")
                        nc.tensor.matmul(pr[:, :w_], lhsT=pshuf, rhs=kb[:, sl_],
                                         start=True, stop=True)
                        t1 = rope_tmp.tile([128, 512], bf16, tag="ropet")
                        nc.vector.tensor_mul(out=t1[:, :w_], in0=pr[:, :w_],
                                             in1=msink[:, sl_])
                        nc.vector.tensor_add(out=kt_t[:, sl_], in0=u[:, sl_],
                                             in1=t1[:, :w_])

                qT.append(qt_t)
                kT.append(kt_t)
                return [c0, c1, c2, c3]

            def attn_front(mt, qt):
                """logits (PE) + exp (Act) + mask-mult (DVE) -> ET."""
                ps_l2 = []
                for hh in range(2):
                    hr = hh * 64
                    ps_l = psl.tile([128, 384], f32, tag="psl")
                    for j in range(3):
                        nc.tensor.matmul(
                            ps_l[:, j * 128:(j + 1) * 128],
                            lhsT=kT[mt][hr:hr + 64, (qt + j) * 128:(qt + j + 1) * 128],
                            rhs=qT[mt][hr:hr + 64, qt * 128:(qt + 1) * 128],
                            start=(j == 0), stop=(j == 2))
                    ps_l2.append(ps_l)
                Eb = at.tile([128, 2, 3, 128], bf16, tag="Eb")
                for hh in range(2):
                    nc.scalar.activation(out=Eb[:, hh, :, :], in_=ps_l2[hh],
                                         func=AF.Exp, scale=float(HD) ** -0.5,
                                         bias=esh_t)
                ET = at.tile([128, 2, 3, 128], f8, tag="ET")
                for hh in range(2):
                    nc.gpsimd.tensor_mul(out=ET[:, hh, :, :], in0=Eb[:, hh, :, :],
                                         in1=masks[qt])
                return (mt, qt, ET)

            def attn_back(ctx):
                """sums + AV (PE), then normalize into o28 (DVE)."""
                mt, qt, ET = ctx
                ps_s = pss.tile([128, 128], f32, tag="pss")
                ps_o = pso.tile([128, 128], f32, tag="pso")
                # DR dst must sit at partition 0, so head hh=0 uses DoubleRow
                # and hh=1 falls back to plain fp8 matmuls
                nc.tensor.matmul(ps_s[0:64, :], lhsT=ones8d,
                                 rhs=ET[:, 0, 0:2, :],
                                 start=True, stop=False, perf_mode=DR)
                nc.tensor.matmul(ps_s[0:64, :], lhsT=ones8d[:, 0, :],
                                 rhs=ET[:, 0, 2, :], start=False, stop=True)
                for j in range(3):
                    nc.tensor.matmul(ps_s[64:128, :], lhsT=ones8d[:, 0, :],
                                     rhs=ET[:, 1, j, :],
                                     start=(j == 0), stop=(j == 2))
                h0 = 2 * mt
                nc.tensor.matmul(ps_o[0:64, :],
                                 lhsT=v8[:, qt:qt + 2, h0 * 64:h0 * 64 + 64],
                                 rhs=ET[:, 0, 0:2, :],
                                 start=True, stop=False, perf_mode=DR)
                nc.tensor.matmul(ps_o[0:64, :],
                                 lhsT=v8[:, qt + 2, h0 * 64:h0 * 64 + 64],
                                 rhs=ET[:, 0, 2, :], start=False, stop=True)
                h1 = 2 * mt + 1
                for j in range(3):
                    nc.tensor.matmul(ps_o[64:128, :],
                                     lhsT=v8[:, qt + j, h1 * 64:h1 * 64 + 64],
                                     rhs=ET[:, 1, j, :],
                                     start=(j == 0), stop=(j == 2))
                rec = rope_tmp.tile([128, 128], f32, tag="rec")
                nc.vector.reciprocal(out=rec, in_=ps_s)
                nc.vector.tensor_mul(out=o28[mt // 2][:, mt % 2, qt * 128:(qt + 1) * 128],
                                     in0=ps_o, in1=rec)

            x2_list = [None] * NQT
            mv2_list = [None] * NQT
            y2_list = [None] * NQT

            def outproj_chunk(qt):
                def f():
                    x2 = x2_pool.tile([128, D], bf16, tag="x2")
                    for half in range(2):
                        ps = psb.tile([128, 512], f32, tag="psqk")
                        for cn in range(2):
                            reg = ps[:, cn * 256:(cn + 1) * 256]
                            for p in range(4):
                                nc.tensor.matmul(
                                    reg,
                                    lhsT=o28[p][:, :, qt * 128:(qt + 1) * 128],
                                    rhs=wo8[:, p, :,
                                            half * 512 + cn * 256:half * 512 + (cn + 1) * 256],
                                    start=(p == 0 and cn == 0), stop=(p == 3),
                                    perf_mode=DR)
                        sl = slice(half * 512, (half + 1) * 512)
                        nc.vector.tensor_add(out=x2[:, sl], in0=ps,
                                             in1=x_tiles[2 + qt][:, sl])
                    x2_list[qt] = x2
                    mv2_list[qt] = ln_stats(x2, ln_tmp2)
                return f

            def lnfin_chunk(qt):
                def f():
                    y2 = y2_pool.tile([128, D], bf16, tag="y2")
                    ln_norm(x2_list[qt], y2, mv2_list[qt], ln_tmp2)
                    y2_list[qt] = y2
                return f

            issue_wqk(0)
            issue_wqk(1)
            issue_wqk(2)
            chunks = proj_chunks(0)
            for c in chunks:
                c()
            fill_plan = {
                (7, 1): [outproj_chunk(0), lnfin_chunk(0)],
                (7, 2): [outproj_chunk(1), lnfin_chunk(1)],
                (7, 3): [outproj_chunk(2), lnfin_chunk(2)],
            }
            ctx = None
            for mt in range(8):
                if 3 <= mt + 3 < 8:
                    issue_wqk(mt + 3)
                if mt + 1 < 8:
                    nxt = proj_chunks(mt + 1)
                for qt in range(NQT):
                    nctx = attn_front(mt, qt)
                    if ctx is not None:
                        attn_back(ctx)
                    if mt + 1 < 8:
                        nxt[qt]()
                    else:
                        for fl in fill_plan.get((mt, qt), []):
                            fl()
                    ctx = nctx
            attn_back(ctx)
            outproj_chunk(NQT - 1)()
            lnfin_chunk(3)()

        # ====== phases D: y2^T triplet interleaved with MLP gate/up ======
        H8 = []
        H8s = []
        with ExitStack() as ph:
            pst2 = ph.enter_context(tc.tile_pool(name="pst2", bufs=4, space="PSUM"))
            wgu_pool = ph.enter_context(tc.tile_pool(name="wgu", bufs=3))
            psg = ph.enter_context(tc.tile_pool(name="psg", bufs=4, space="PSUM"))
            gu_tmp = ph.enter_context(tc.tile_pool(name="gu_tmp", bufs=4))

            # y2^T split by query halves: a = tokens 0:256 (qt 0/1), b = 256:512
            y2T8a = [y2T_pool.tile([128, 2, 256], f8, name="y2T8a", tag="y2T8a")
                     for _ in range(4)]
            y2T8b = [y2T_pool.tile([128, 2, 256], f8, name="y2T8b", tag="y2T8b")
                     for _ in range(4)]
            y2T8sa = [y2T_pool.tile([128, 2, 256], f8, name="y2T8sa", tag="y2T8sa")
                      for _ in range(4)]
            y2T8sb = [y2T_pool.tile([128, 2, 256], f8, name="y2T8sb", tag="y2T8sb")
                      for _ in range(4)]
            y2Tr8a = [y2T_pool.tile([128, 2, 256], f8, name="y2Tr8a", tag="y2Tr8a")
                      for _ in range(4)] if GU_THIRD else None
            y2Tr8b = [y2T_pool.tile([128, 2, 256], f8, name="y2Tr8b", tag="y2Tr8b")
                      for _ in range(4)] if GU_THIRD else None
            gu_terms = [((y2T8a, y2T8b), 0), ((y2T8sa, y2T8sb), 1)] + \
                ([((y2Tr8a, y2Tr8b), 0)] if GU_THIRD else [])
            nterm = len(gu_terms)

            for pair in range(16):
                H8.append(hh_pool.tile([128, 2, CS], f8, name="H8", tag="hh"))
                H8s.append(hh_pool.tile([128, 2, CS], f8, name="H8s", tag="hhs"))

            def y2t_pass(half, hi_l, s_l, r_l):
                """Transpose qt pair (2*half, 2*half+1) for all 8 dtiles and
                ship the corresponding token-half fp8 triplet."""
                pts = {}
                for pair in range(4):
                    pts[pair] = pst2.tile([128, 2, 2, 128], bf16, name="pt2",
                                          tag="pst2b")
                for qi in range(2):
                    qt = half * 2 + qi
                    for pair in range(4):
                        for di in range(2):
                            dtl = pair * 2 + di
                            nc.tensor.transpose(
                                pts[pair][:, di, qi, :],
                                y2_list[qt][:, dtl * 128:(dtl + 1) * 128],
                                identb)
                for pair in range(4):
                    pt = pts[pair]
                    dst_hi = hi_l[pair][:, :, :]
                    nc.scalar.activation(out=dst_hi, in_=pt, func=AF.Identity)
                    nc.gpsimd.tensor_scalar_mul(out=s_l[pair][:, :, :],
                                                in0=dst_hi, scalar1=1.0 / 16)
                    if GU_THIRD:
                        nc.vector.tensor_tensor(out=r_l[pair][:, :, :],
                                                in0=pt, in1=dst_hi,
                                                op=OP.subtract)

            wgu_tiles = {}
            wd_tiles = {}

            def issue_wd(pair):
                w = wd_pool.tile([128, 2, 2, 1024], f8, name="wd", tag="wd")
                eng = nc.sync if pair % 2 == 0 else nc.gpsimd
                eng.dma_start(out=w, in_=wd_d[pair])
                wd_tiles[pair] = w

            def gu_mt_pass(mt, ps_pair, p, cns=(0, 1)):
                """K-pair accumulation pass of gate+up for f-block mt over the
                given column halves (cn 0 reads the a tiles, 1 the b)."""
                w = wgu_tiles[mt]
                for gi in range(2):
                    ps = ps_pair[gi]
                    for cn in cns:
                        reg = ps[:, cn * 256:(cn + 1) * 256]
                        for ti, (act, hl) in enumerate(gu_terms):
                            last = (p == 3 and ti == nterm - 1)
                            nc.tensor.matmul(
                                reg,
                                lhsT=w[:, gi, hl, p, :, :],
                                rhs=act[cn][p][:, :, :],
                                start=(p == 0 and ti == 0 and cn == 0),
                                stop=(last and not (has_bg and gi == 0)),
                                perf_mode=DR)

            def gu_mt_finish(mt, ps_pair):
                if has_bg:
                    for cn in range(2):
                        nc.tensor.matmul(
                            ps_pair[0][:, cn * 256:(cn + 1) * 256],
                            lhsT=bg_sb[:, mt * 128:(mt + 1) * 128],
                            rhs=ones_row[:, cn * 256:(cn + 1) * 256],
                            start=False, stop=True)
                U = gu_tmp.tile([128, CS], bf16, tag="U")
                nc.scalar.activation(out=U, in_=ps_pair[1], func=AF.Silu,
                                     bias=bu_sb[:, mt:mt + 1], scale=1.0)
                h8_dst = H8[mt // 2][:, mt % 2, :]
                nc.vector.tensor_mul(out=h8_dst, in0=ps_pair[0], in1=U)
                nc.gpsimd.tensor_scalar_mul(out=H8s[mt // 2][:, mt % 2, :],
                                            in0=h8_dst, scalar1=0.125)

            def new_gu_ps():
                return [psg.tile([128, CS], f32, name="psgu", tag="psgu")
                        for _ in range(2)]

            def issue_wgu(mt):
                pool = wgu0_pool if mt < 2 else wgu_pool
                w = pool.tile([128, 2, 2, 4, 2, 128], f8, name="wgu", tag="wgu")
                eng = nc.sync if mt % 2 == 0 else nc.gpsimd
                eng.dma_start(out=w, in_=wgu_d[mt])
                wgu_tiles[mt] = w

            # mt 0/1: pair passes interleaved with the y2T wave production so
            # PE stays fed while the transposes/copies stream out
            issue_wgu(0)
            issue_wgu(1)
            ps0, ps1 = new_gu_ps(), new_gu_ps()
            y2t_pass(0, y2T8a, y2T8sa, y2Tr8a)
            y2t_pass(1, y2T8b, y2T8sb, y2Tr8b)
            for p in range(4):
                gu_mt_pass(0, ps0, p, cns=(0,))
                gu_mt_pass(1, ps1, p, cns=(0,))
            for p in range(4):
                gu_mt_pass(0, ps0, p, cns=(1,))
                gu_mt_pass(1, ps1, p, cns=(1,))
            gu_mt_finish(0, ps0)
            gu_mt_finish(1, ps1)
            wgu_tiles.pop(0)
            wgu_tiles.pop(1)

            issue_wgu(2)
            for mt in range(2, 32):
                if mt + 1 < 32:
                    issue_wgu(mt + 1)
                if mt in (19, 21, 23, 25):
                    issue_wd((mt - 19) // 2)
                w = wgu_tiles[mt]
                psm = new_gu_ps()
                for p in range(4):
                    gu_mt_pass(mt, psm, p)
                gu_mt_finish(mt, psm)
                wgu_tiles.pop(mt)

        # ====== phase E: down proj (x4 weights) + residual + store ======
        with ExitStack() as ph:
            psd = ph.enter_context(tc.tile_pool(name="psd", bufs=8, space="PSUM"))
            out_pool = ph.enter_context(tc.tile_pool(name="outp", bufs=2))
            dn_tmp = ph.enter_context(tc.tile_pool(name="dn_tmp", bufs=2))

            ps_d = [psd.tile([128, 512], f32, name="psd", tag="psd") for _ in range(8)]
            dn_terms = [(H8, 0), (H8s, 1)]

            def dn_finish(tt):
                ot = out_pool.tile([128, D], f32, name="outp", tag="outp")
                for ch3 in range(2):
                    sl = slice(ch3 * 512, (ch3 + 1) * 512)
                    pd = ps_d[tt * 2 + ch3]
                    if has_bd:
                        # bias pre-scaled x4 on host to match the x4 weights
                        nc.tensor.matmul(pd[:, 0:256], lhsT=ones_row[:, 0:128],
                                         rhs=bd_sb[:, ch3 * 512:ch3 * 512 + 256],
                                         start=False, stop=True)
                        nc.tensor.matmul(pd[:, 256:512], lhsT=ones_row[:, 0:128],
                                         rhs=bd_sb[:, ch3 * 512 + 256:(ch3 + 1) * 512],
                                         start=False, stop=True)
                    tmp = dn_tmp.tile([128, 512], f32, name="dntmp", tag="dntmp")
                    nc.scalar.activation(out=tmp, in_=pd, func=AF.Identity,
                                         scale=0.25)
                    nc.vector.tensor_add(out=ot[:, sl], in0=tmp,
                                         in1=x2_list[tt][:, sl])
                eng2 = nc.sync if tt % 2 == 0 else nc.gpsimd
                eng2.dma_start(out=out_d[tt * 128:(tt + 1) * 128, :], in_=ot)

            for pair in range(16):
                if pair >= 1 and pair + 3 < 16:
                    issue_wd(pair + 3)
                w = wd_tiles.pop(pair)
                for tt in range(NQT):
                    for ti, (act, hl) in enumerate(dn_terms):
                        for cn in range(4):
                            reg = ps_d[tt * 2 + cn // 2][:, (cn % 2) * 256:
                                                         (cn % 2 + 1) * 256]
                            nc.tensor.matmul(
                                reg,
                                lhsT=act[pair][:, :, tt * 128:(tt + 1) * 128],
                                rhs=w[:, hl, :, cn * 256:(cn + 1) * 256],
                                start=(pair == 0 and ti == 0 and cn % 2 == 0),
                                stop=(pair == 15 and ti == 1 and not has_bd),
                                perf_mode=DR)
                    if pair == 15:
                        dn_finish(tt)
        mlp_scope.close()
        wqkp_scope.close()
        qkv_scope.close()
        mid_scope.close()

    nc.compile()
    return nc


def prep_inputs(x, w_qkv, w_out, g1, b1, g2, b2, w_gate, b_gate, w_up, b_up,
                w_down, b_down):
    """Host-side: fold LN params, fp8-split weights, pre-tile, build per-core
    tensors."""
    import ml_dtypes
    f32 = np.float32
    bf16 = ml_dtypes.bfloat16
    f8 = ml_dtypes.float8_e4m3

    def split8(w, s=16.0):
        hi = w.astype(f8)
        lo = ((w - hi.astype(f32)) * s).astype(f8)
        return hi, lo

    wqkv_f = (w_qkv * g1[:, None]).astype(f32)
    bqkv = (b1 @ w_qkv).astype(f32)

    def qk_tile(w):  # [D, 1024] -> [mt, p, pair, i, m] fp8 pieces
        hi, lo = split8(w)
        t = lambda a: np.ascontiguousarray(
            a.reshape(4, 2, 128, 8, 128).transpose(3, 2, 0, 1, 4))
        return t(hi), t(lo)

    qhi, qlo = qk_tile(wqkv_f[:, :D])
    khi, klo = qk_tile(wqkv_f[:, D:2 * D])
    wqk = np.ascontiguousarray(
        np.stack([qhi, qlo, khi, klo], axis=2))  # [8,128,4,4,2,128]

    def mv_tile(w):  # [D, 1024] -> [p, pair, i, n]
        return w.reshape(4, 2, 128, 1024).transpose(2, 0, 1, 3)

    vhi, vlo = split8(wqkv_f[:, 2 * D:])
    wv = np.ascontiguousarray(np.stack([mv_tile(vhi), mv_tile(vlo)], axis=1))
    wo = np.ascontiguousarray(mv_tile(w_out.astype(f32).astype(f8)))

    def gu_tile(w):  # [D, F] -> [mt, p, hi/lo, pair, i, m]
        hi, lo = split8(w)
        t = lambda a: a.reshape(4, 2, 128, 32, 128).transpose(3, 2, 0, 1, 4)
        return np.stack([t(hi), t(lo)], axis=2)  # [32,128,2,4,2,128]

    wg_f = (w_gate * g2[:, None]).astype(f32)
    wu_f = (w_up * g2[:, None]).astype(f32)
    wgu = np.ascontiguousarray(
        np.stack([gu_tile(wg_f), gu_tile(wu_f)], axis=2))  # [32,128,2,2,4,2,128]

    wd_f = w_down.astype(f32)
    wd_hi = (4.0 * wd_f).astype(f8)
    wd_lo = (32.0 * (wd_f - wd_hi.astype(f32) / 4.0)).astype(f8)
    t_wd = lambda a: a.reshape(16, 2, 128, 1024).transpose(0, 2, 1, 3)
    wd = np.ascontiguousarray(np.stack([t_wd(wd_hi), t_wd(wd_lo)], axis=2))

    bqk_pt = bqkv[:2048].reshape(16, 128).T                       # [p, t]
    bu_pt = (b_up + b2 @ w_up).astype(f32).reshape(32, 128).T
    cbf = np.ascontiguousarray(
        np.concatenate([bqk_pt, bu_pt], axis=1)).astype(f32)      # [128, 48]

    bg_row = (b_gate + b2 @ w_gate).astype(f32).reshape(1, F).astype(bf16)
    bv_row = bqkv[2048:].reshape(1, D).astype(bf16)
    bd_row = (4.0 * b_down).reshape(1, D).astype(bf16)

    # rotate-half permutation (sign folded into sin tables)
    pshuf = np.zeros((128, 128), f32)
    for m in range(128):
        base = (m // 64) * 64
        r = m % 64
        sig = base + (r + 32) % 64
        pshuf[sig, m] = 1.0
    pshuf = pshuf.astype(bf16)

    half = HD // 2
    inv_freq = 1.0 / (10000.0 ** (np.arange(half, dtype=np.float64) / half))

    def rope_tables(pos):
        t = np.maximum(pos, 0).astype(np.float64)
        freqs = np.outer(t, inv_freq)
        emb = np.concatenate([freqs, freqs], 1)
        c = np.cos(emb).T.astype(f32)
        s = np.sin(emb).T.astype(f32)
        ms = s.copy()
        ms[:32] = -ms[:32]
        return (np.ascontiguousarray(np.vstack([c, c])),
                np.ascontiguousarray(np.vstack([ms, ms])))

    common = {"wqk": wqk, "wv": wv, "wo": wo, "wgu": wgu, "wd": wd,
              "bv": bv_row, "bd": bd_row, "bg": bg_row, "cbf": cbf}

    in_maps = []
    for c in range(NCORES):
        b, chunk = c // CH, c % CH
        q0 = chunk * CS
        lo = q0 - WIN
        xh = np.zeros((HT, D), f32)
        src_lo = max(0, lo)
        xh[src_lo - lo:] = x[b, src_lo:q0 + CS]
        xh = xh.astype(bf16)
        pos_k = np.arange(lo, q0 + CS)
        cosk_a, sink_a = rope_tables(pos_k)
        # transposed multiplicative mask [r, qt, j, c]:
        #   key j_g = lo + (qt+j)*128 + r ; query i = q0 + qt*128 + c
        r_i = np.arange(128)[:, None, None, None]
        qt_i = np.arange(NQT)[None, :, None, None]
        j_i = np.arange(3)[None, None, :, None]
        c_i = np.arange(128)[None, None, None, :]
        jg = lo + (qt_i + j_i) * 128 + r_i
        gi = q0 + qt_i * 128 + c_i
        valid = (jg <= gi) & (gi - jg <= WIN) & (jg >= 0)
        maskT = valid.astype(f32).reshape(128, NQT * 3 * 128).astype(bf16)
        cbb = np.concatenate(
            [cosk_a.astype(bf16), sink_a.astype(bf16), maskT, pshuf], axis=1)
        in_maps.append(dict(common, xh=xh, cbb=np.ascontiguousarray(cbb)))
    return in_maps


_PROG = {}


def kernel(**inputs):
    from concourse.bass_utils import run_bass_kernel_spmd

    inputs = {k: np.asarray(v, dtype=np.float32) for k, v in inputs.items()}
    in_maps = prep_inputs(**inputs)
    flags = (bool(np.any(inputs["b1"] @ inputs["w_qkv"][:, 2048:])),
             bool(np.any(inputs["b_gate"] + inputs["b2"] @ inputs["w_gate"])),
             bool(np.any(inputs["b_down"])))
    if flags not in _PROG:
        _PROG[flags] = build_program(has_bv=flags[0], has_bg=flags[1],
                                     has_bd=flags[2])
    nc = _PROG[flags]
    res = run_bass_kernel_spmd(nc, in_maps, core_ids=list(range(NCORES)))
    out = np.zeros((B, S, D), np.float32)
    for c in range(NCORES):
        b, chunk = c // CH, c % CH
        out[b, chunk * CS:(chunk + 1) * CS] = res.results[c]["out"]
    return out


# revision 37
# speedup vs baseline: 1.0253x; 1.0253x over previous
"""Trainium2 Bass kernel for AdvancedTransformerEncoderBlock (fp8 DoubleRow).

Sharding: token-parallel across 8 cores (B=2 x 4 seq chunks of 512), each core
recomputes a 256-token K/V halo -> zero collectives.

Precision plan (validated vs fp32 reference, rel_err ~= 0.015):
  - qkv proj:   fp8e4 DoubleRow, weights split hi+lo(x16), activation split
                hi + hi/16 + residual  (3 passes, 4x per-pass speedup)
  - attention:  bf16 (transposed-logits flow: logits land [keys, queries] in
                PSUM; exp on Act; band mask folded into the PSUM->SBUF copy as
                a 0/1 multiply; softmax sums via ones[128,64] matmul so the
                per-query denominators arrive broadcast across partitions;
                normalize folded into the o2 copy)
  - out proj:   fp8e4 DoubleRow single-pass (o2/wo plain fp8)
  - gate/up:    like qkv (3 passes)
  - down proj:  weights split fp8(4w) + fp8(32*res), H plain fp8 + H/8 copy;
                the 4x weight prescale (keeps wd out of fp8 subnormals) is
                undone by a 0.25 scale folded into the PSUM->SBUF copy
PSUM accumulation stays fp32, residual stream stays fp32.
RoPE rotate-half runs as a PE permutation matmul.
Attention runs one query-tile ahead on logits so exp/mask latency hides under
sums/AV of the previous tile plus the interleaved projection fillers.
"""

import numpy as np

B, S, D, F, H, HD = 2, 2048, 1024, 4096, 16, 64
WIN = 256
NCORES = 8
CH = 4           # chunks per batch
CS = S // CH     # 512 tokens per chunk (queries)
HT = CS + WIN    # 768 tokens incl. halo (keys/values)
NQT = CS // 128  # 4 query tiles
EPS = 1e-5
QKV_THIRD = True   # include activation-residual pass in qkv proj
GU_THIRD = True    # include activation-residual pass in gate/up


def build_program(has_bv=False, has_bg=False, has_bd=False):
    import concourse.bass as bass
    import concourse.bacc as bacc_mod
    import concourse.tile as tile
    import concourse.mybir as mybir
    from concourse.masks import make_identity
    from contextlib import ExitStack

    dt = mybir.dt
    f32, bf16, f8 = dt.float32, dt.bfloat16, dt.float8e4
    AF = mybir.ActivationFunctionType
    OP = mybir.AluOpType
    DR = mybir.MatmulPerfMode.DoubleRow

    nc = bacc_mod.Bacc()
    Pf = lambda name, shape: nc.declare_dram_parameter(name, list(shape), f32, isOutput=False)
    Pb = lambda name, shape: nc.declare_dram_parameter(name, list(shape), bf16, isOutput=False)
    P8 = lambda name, shape: nc.declare_dram_parameter(name, list(shape), f8, isOutput=False)

    xh_d = Pb("xh", (HT, D))
    wqk_d = P8("wqk", (8, 128, 4, 4, 2, 128))   # [mt][p][qhi,qlo,khi,klo][pair][i][m]
    wv_d = P8("wv", (128, 2, 4, 2, 1024))       # [p][hi/lo][pair][i][n]
    wo_d = P8("wo", (128, 4, 2, 1024))          # [p][pair][i][n]
    wgu_d = P8("wgu", (32, 128, 2, 2, 4, 2, 128))  # [mt][p][g/u][hi/lo][pair][i][m]
    wd_d = P8("wd", (16, 128, 2, 2, 1024))      # [pair][p][hi/lo][i][n]
    bv_d = Pb("bv", (1, D))
    bd_d = Pb("bd", (1, D))
    bg_d = Pb("bg", (1, F))
    cbf_d = Pf("cbf", (128, 48))                # bqk [:,0:16], bu [:,16:48]
    cbb_d = Pb("cbb", (128, 3200))
    out_d = nc.declare_dram_parameter("out", [CS, D], f32, isOutput=True)

    with tile.TileContext(nc) as tc, ExitStack() as top:
        const = top.enter_context(tc.tile_pool(name="const", bufs=1))

        # x tiles first: their DMAs head the queue so LN/transposes start early
        x_pool = top.enter_context(tc.tile_pool(name="x", bufs=6))
        x_tiles = []
        for tt in range(6):
            xt = x_pool.tile([128, D], bf16, tag="xt")
            eng = nc.sync if tt % 2 == 0 else nc.gpsimd
            if tt == 0:
                eng.dma_start(out=xt[:, 0:512], in_=xh_d[0:128, 0:512])
                eng.dma_start(out=xt[:, 512:1024], in_=xh_d[0:128, 512:1024])
            else:
                eng.dma_start(out=xt, in_=xh_d[tt * 128:(tt + 1) * 128, :])
            x_tiles.append(xt)

        # ---- constants ----
        cbf = const.tile([128, 48], f32, tag="cbf")
        nc.sync.dma_start(out=cbf, in_=cbf_d[:, :])
        cbb = const.tile([128, 3200], bf16, tag="cbb")
        nc.gpsimd.dma_start(out=cbb, in_=cbb_d[:, :])
        bqk_sb = cbf[:, 0:16]
        bu_sb = cbf[:, 16:48]
        cosk = cbb[:, 0:768]
        msink = cbb[:, 768:1536]
        cosq = cbb[:, WIN:768]
        msinq = cbb[:, 768 + WIN:1536]
        masks = [cbb[:, 1536 + qt * 384:1536 + (qt + 1) * 384] for qt in range(NQT)]
        pshuf = cbb[:, 3072:3200]

        identb = const.tile([128, 128], bf16, tag="identb")
        make_identity(nc, identb)
        ones8d = const.tile([128, 2, 64], f8, tag="ones8d")
        nc.vector.memset(ones8d, 1.0)
        ones_row = const.tile([1, 512], bf16, tag="ones_row")
        nc.vector.memset(ones_row, 1.0)
        eps_t = const.tile([128, 1], f32, tag="eps")
        nc.vector.memset(eps_t, EPS)
        esh_t = const.tile([128, 1], f32, tag="esh")
        nc.vector.memset(esh_t, -3.0)
        if has_bv:
            bv_sb = const.tile([1, D], bf16, tag="bv")
            nc.sync.dma_start(out=bv_sb, in_=bv_d[:, :])
        if has_bd:
            bd_sb = const.tile([1, D], bf16, tag="bd")
            nc.sync.dma_start(out=bd_sb, in_=bd_d[:, :])
        if has_bg:
            bg_sb = const.tile([1, F], bf16, tag="bg")
            nc.sync.dma_start(out=bg_sb, in_=bg_d[:, :])

        # ---- persistent activation pools ----
        x2_pool = top.enter_context(tc.tile_pool(name="x2", bufs=4))
        y2T_pool = top.enter_context(tc.tile_pool(name="y2T", bufs=4))
        o2_pool = top.enter_context(tc.tile_pool(name="o2", bufs=4))

        def ln_stats(src, tmp_pool):
            stats = tmp_pool.tile([128, 2, 6], f32, tag="lnstats")
            mv = tmp_pool.tile([128, 2], f32, tag="lnmv")
            for sg in range(2):
                nc.vector.bn_stats(out=stats[:, sg, :], in_=src[:, sg * 512:(sg + 1) * 512])
            nc.vector.bn_aggr(out=mv, in_=stats)
            return mv

        def ln_norm(src, dst, mv, tmp_pool):
            rs = tmp_pool.tile([128, 1], f32, tag="lnrs")
            nc.scalar.activation(out=rs, in_=mv[:, 1:2], func=AF.Sqrt,
                                 bias=eps_t, scale=1.0)
            nc.vector.reciprocal(out=rs, in_=rs)
            nb = tmp_pool.tile([128, 1], f32, tag="lnnb")
            nc.vector.tensor_scalar(out=nb, in0=mv[:, 0:1], scalar1=rs,
                                    scalar2=-1.0, op0=OP.mult, op1=OP.mult)
            nc.scalar.activation(out=dst, in_=src, func=AF.Identity,
                                 bias=nb, scale=rs)

        def layernorm(src, dst, tmp_pool):
            ln_norm(src, dst, ln_stats(src, tmp_pool), tmp_pool)

        # mid-lived: x2-LN workspace + y2 (read by the late y2T transposes)
        mid_scope = ExitStack()
        ln_tmp2 = mid_scope.enter_context(tc.tile_pool(name="ln_tmp2", bufs=3))
        y2_pool = mid_scope.enter_context(tc.tile_pool(name="y2", bufs=4))

        qkv_scope = ExitStack()
        yT_pool = qkv_scope.enter_context(tc.tile_pool(name="yT", bufs=4))
        wo_pool = qkv_scope.enter_context(tc.tile_pool(name="wo", bufs=1))
        wo8 = wo_pool.tile([128, 4, 2, 1024], f8, tag="wo8")
        qT_pool = qkv_scope.enter_context(tc.tile_pool(name="qT", bufs=3))
        kT_pool = qkv_scope.enter_context(tc.tile_pool(name="kT", bufs=3))
        vb_pool = qkv_scope.enter_context(tc.tile_pool(name="vb", bufs=6))


        # y^T pair tiles split by token halves so consumers start after the
        # first three LN outputs: a = tokens 0:384, b = 384:768
        HH = HT // 2
        yT8a = [yT_pool.tile([128, 2, HH], f8, name="yT8a", tag="yT8a") for _ in range(4)]
        yT8b = [yT_pool.tile([128, 2, HH], f8, name="yT8b", tag="yT8b") for _ in range(4)]
        yT8sa = [yT_pool.tile([128, 2, HH], f8, name="yT8sa", tag="yT8sa") for _ in range(4)]
        yT8sb = [yT_pool.tile([128, 2, HH], f8, name="yT8sb", tag="yT8sb") for _ in range(4)]
        yTr8a = [yT_pool.tile([128, 2, HH], f8, name="yTr8a", tag="yTr8a")
                 for _ in range(4)] if QKV_THIRD else None
        yTr8b = [yT_pool.tile([128, 2, HH], f8, name="yTr8b", tag="yTr8b")
                 for _ in range(4)] if QKV_THIRD else None

        # =========== phase A: LN1 -> y -> y^T fp8 triplet ===========
        with ExitStack() as ph:
            ln_tmp = ph.enter_context(tc.tile_pool(name="ln_tmp", bufs=6))
            y_pool = ph.enter_context(tc.tile_pool(name="y", bufs=6))
            pst = ph.enter_context(tc.tile_pool(name="pst", bufs=8, space="PSUM"))

            ys = []
            for tt in range(6):
                y = y_pool.tile([128, D], bf16, tag="y")
                layernorm(x_tiles[tt], y, ln_tmp)
                ys.append(y)
            # tt-outer with half-granular copies: the a-half (tokens 0:384)
            # ships as soon as the first three LN outputs exist
            pts = [pst.tile([128, 6, 128], bf16, name="pt", tag="pst")
                   for _ in range(8)]
            for half, (hi_l, s_l, r_l) in enumerate(
                    [(yT8a, yT8sa, yTr8a), (yT8b, yT8sb, yTr8b)]):
                for tt in range(half * 3, half * 3 + 3):
                    for dtl in range(8):
                        nc.tensor.transpose(pts[dtl][:, tt, :],
                                            ys[tt][:, dtl * 128:(dtl + 1) * 128],
                                            identb)
                for dtl in range(8):
                    pt = pts[dtl][:, half * 3:half * 3 + 3, :]
                    dst_hi = hi_l[dtl // 2][:, dtl % 2, :]
                    nc.scalar.activation(out=dst_hi, in_=pt, func=AF.Identity)
                    nc.gpsimd.tensor_scalar_mul(out=s_l[dtl // 2][:, dtl % 2, :],
                                                in0=dst_hi, scalar1=1.0 / 16)
                    if QKV_THIRD:
                        nc.vector.tensor_tensor(out=r_l[dtl // 2][:, dtl % 2, :],
                                                in0=pt, in1=dst_hi, op=OP.subtract)

        # late-lived pools opened after phase A so their space reuses the LN
        # workspace; DMAs for v/out-proj weights head the queue here
        wqkp_scope = ExitStack()
        wqk_pool = wqkp_scope.enter_context(tc.tile_pool(name="wqkp", bufs=3))
        mlp_scope = ExitStack()
        hh_pool = mlp_scope.enter_context(tc.tile_pool(name="hh", bufs=16))
        wd_pool = mlp_scope.enter_context(tc.tile_pool(name="wd", bufs=4))
        wgu0_pool = mlp_scope.enter_context(tc.tile_pool(name="wgu0", bufs=2))
        vw_scope = ExitStack()
        wv_pool = vw_scope.enter_context(tc.tile_pool(name="wv", bufs=1))
        wv8 = wv_pool.tile([128, 2, 4, 2, 1024], f8, tag="wv8")
        for p_ in range(4):
            eng = nc.sync if p_ % 2 == 0 else nc.gpsimd
            eng.dma_start(out=wv8[:, :, p_, :, :], in_=wv_d[:, :, p_, :, :])
        nc.gpsimd.dma_start(out=wo8, in_=wo_d[:, :, :, :])

        qkv_terms = [(yT8a, yT8b, 0), (yT8sa, yT8sb, 1)] + \
            ([(yTr8a, yTr8b, 0)] if QKV_THIRD else [])

        def ytok(term, lo, hi):
            """AP for token range [lo, hi) of a qkv term (within one half)."""
            a_l, b_l, _ = term
            if hi <= HH:
                return lambda p: a_l[p][:, :, lo:hi]
            assert lo >= HH
            return lambda p: b_l[p][:, :, lo - HH:hi - HH]

        # =========== phase B: v projection (fp8 DR, pair-outer) ===========
        with ExitStack() as ph:
            psv = ph.enter_context(tc.tile_pool(name="psv", bufs=6, space="PSUM"))
            v8 = vb_pool.tile([128, 6, D], f8, tag="vbf")
            nterm = len(qkv_terms)
            for chv in range(2):
                pv = [psv.tile([128, 512], f32, name="psv", tag="psv") for _ in range(6)]
                for ti, term in enumerate(qkv_terms):
                    hl = term[2]
                    for p in range(4):
                        for tt in range(6):
                            lsrc = ytok(term, tt * 128, (tt + 1) * 128)(p)
                            for cn in range(2):
                                reg = pv[tt][:, cn * 256:(cn + 1) * 256]
                                last = (p == 3 and ti == nterm - 1)
                                nc.tensor.matmul(
                                    reg,
                                    lhsT=lsrc,
                                    rhs=wv8[:, hl, p, :,
                                            chv * 512 + cn * 256:chv * 512 + (cn + 1) * 256],
                                    start=(ti == 0 and p == 0 and cn == 0),
                                    stop=(last and not has_bv), perf_mode=DR)
                if has_bv:
                    for tt in range(6):
                        for cn in range(2):
                            nc.tensor.matmul(pv[tt][:, cn * 256:(cn + 1) * 256],
                                             lhsT=ones_row[:, 0:128],
                                             rhs=bv_sb[:, chv * 512 + cn * 256:
                                                       chv * 512 + (cn + 1) * 256],
                                             start=False, stop=True)
                for tt in range(6):
                    sl = slice(chv * 512, (chv + 1) * 512)
                    if tt % 2 == 0:
                        nc.scalar.copy(out=v8[:, tt, sl], in_=pv[tt])
                    else:
                        nc.vector.tensor_copy(out=v8[:, tt, sl], in_=pv[tt])
        vw_scope.close()

        # ==== phase C: q/k proj + RoPE pipelined with attention (flow B) ====
        qT, kT = [], []
        with ExitStack() as ph:
            psb = ph.enter_context(tc.tile_pool(name="psb", bufs=3, space="PSUM"))
            rope_tmp = ph.enter_context(tc.tile_pool(name="rope_tmp", bufs=2))
            at = ph.enter_context(tc.tile_pool(name="at", bufs=3))
            psl = ph.enter_context(tc.tile_pool(name="psl", bufs=2, space="PSUM"))
            pss = ph.enter_context(tc.tile_pool(name="pss", bufs=1, space="PSUM"))
            pso = ph.enter_context(tc.tile_pool(name="pso", bufs=2, space="PSUM"))

            o28 = [o2_pool.tile([128, 2, CS], f8, name="o28", tag="o28")
                   for _ in range(4)]

            wqk_tiles = {}

            def issue_wqk(mt):
                w = wqk_pool.tile([128, 4, 4, 2, 128], f8, name="wqk", tag="wqk")
                nc.sync.dma_start(out=w, in_=wqk_d[mt])
                wqk_tiles[mt] = w

            def proj_chunks(mt):
                w = wqk_tiles.pop(mt)
                qt_t = qT_pool.tile([128, CS], bf16, tag="qT")
                kt_t = kT_pool.tile([128, HT], bf16, tag="kT")
                st = {}

                def emit_proj(ps, wbase, tok_lo, tok_hi):
                    cuts = sorted({tok_lo, tok_hi}
                                  | {c for c in (HH, 256, 640) if tok_lo < c < tok_hi})
                    chunks = list(zip(cuts[:-1], cuts[1:]))
                    for cn, (c0, c1) in enumerate(chunks):
                        reg = ps[:, c0 - tok_lo:c1 - tok_lo]
                        for ti, term in enumerate(qkv_terms):
                            hl = term[2]
                            for p in range(4):
                                nc.tensor.matmul(
                                    reg,
                                    lhsT=w[:, wbase + hl, p, :, :],
                                    rhs=ytok(term, c0, c1)(p),
                                    start=(ti == 0 and p == 0 and cn == 0),
                                    stop=(ti == len(qkv_terms) - 1 and p == 3),
                                    perf_mode=DR)

                def c0():  # q projection
                    ps = psb.tile([128, CS], f32, tag="psqk")
                    emit_proj(ps, 0, WIN, HT)
                    qb = rope_tmp.tile([128, CS], bf16, tag="ropesrc")
                    nc.scalar.activation(out=qb, in_=ps, func=AF.Identity,
                                         bias=bqk_sb[:, mt:mt + 1], scale=1.0)
                    st["qb"] = qb

                def c1():  # q rope
                    qb = st["qb"]
                    pr = psb.tile([128, 512], f32, tag="psqk")
                    nc.tensor.matmul(pr, lhsT=pshuf, rhs=qb, start=True, stop=True)
                    u = rope_tmp.tile([128, HT], bf16, tag="ropeu")
                    nc.vector.tensor_mul(out=u[:, :CS], in0=qb, in1=cosq)
                    t1 = rope_tmp.tile([128, 512], bf16, tag="ropet")
                    nc.vector.tensor_mul(out=t1, in0=pr, in1=msinq)
                    nc.vector.tensor_add(out=qt_t, in0=u[:, :CS], in1=t1)

                def c2():  # k projection half 0
                    kb = rope_tmp.tile([128, HT], bf16, tag="ropesrck")
                    st["kb"] = kb
                    ps = psb.tile([128, 384], f32, tag="psqk")
                    emit_proj(ps, 2, 0, 384)
                    nc.scalar.activation(out=kb[:, 0:384], in_=ps, func=AF.Identity,
                                         bias=bqk_sb[:, 8 + mt:9 + mt], scale=1.0)

                def c3():  # k projection half 1 + k rope
                    kb = st["kb"]
                    ps = psb.tile([128, 384], f32, tag="psqk")
                    emit_proj(ps, 2, 384, HT)
                    nc.scalar.activation(out=kb[:, 384:768], in_=ps, func=AF.Identity,
                                         bias=bqk_sb[:, 8 + mt:9 + mt], scale=1.0)
                    u = rope_tmp.tile([128, HT], bf16, tag="ropeu")
                    nc.vector.tensor_mul(out=u, in0=kb, in1=cosk)
                    for c in range(2):
                        w_ = 512 if c == 0 else 256
                        sl_ = slice(c * 512, c * 512 + w_)
                        pr = psb.tile([128, 512], f32, tag="psqk")
                        nc.tensor.matmul(pr[:, :w_], lhsT=pshuf, rhs=kb[:, sl_],
                                         start=True, stop=True)
                        t1 = rope_tmp.tile([128, 512], bf16, tag="ropet")
                        nc.vector.tensor_mul(out=t1[:, :w_], in0=pr[:, :w_],
                                             in1=msink[:, sl_])
                        nc.vector.tensor_add(out=kt_t[:, sl_], in0=u[:, sl_],
                                             in1=t1[:, :w_])

                qT.append(qt_t)
                kT.append(kt_t)
                return [c0, c1, c2, c3]

            def attn_front(mt, qt):
                """logits (PE) + exp (Act) + mask-mult (DVE) -> ET."""
                ps_l2 = []
                for hh in range(2):
                    hr = hh * 64
                    ps_l = psl.tile([128, 384], f32, tag="psl")
                    for j in range(3):
                        nc.tensor.matmul(
                            ps_l[:, j * 128:(j + 1) * 128],
                            lhsT=kT[mt][hr:hr + 64, (qt + j) * 128:(qt + j + 1) * 128],
                            rhs=qT[mt][hr:hr + 64, qt * 128:(qt + 1) * 128],
                            start=(j == 0), stop=(j == 2))
                    ps_l2.append(ps_l)
                Eb = at.tile([128, 2, 3, 128], bf16, tag="Eb")
                for hh in range(2):
                    nc.scalar.activation(out=Eb[:, hh, :, :], in_=ps_l2[hh],
                                         func=AF.Exp, scale=float(HD) ** -0.5,
                                         bias=esh_t)
                ET = at.tile([128, 2, 3, 128], f8, tag="ET")
                nc.vector.tensor_mul(out=ET[:, 0, :, :], in0=Eb[:, 0, :, :],
                                     in1=masks[qt])
                nc.gpsimd.tensor_mul(out=ET[:, 1, :, :], in0=Eb[:, 1, :, :],
                                     in1=masks[qt])
                return (mt, qt, ET)

            def attn_back(ctx):
                """sums + AV (PE), then normalize into o28 (DVE)."""
                mt, qt, ET = ctx
                ps_s = pss.tile([128, 128], f32, tag="pss")
                ps_o = pso.tile([128, 128], f32, tag="pso")
                # DR dst must sit at partition 0, so head hh=0 uses DoubleRow
                # and hh=1 falls back to plain fp8 matmuls
                nc.tensor.matmul(ps_s[0:64, :], lhsT=ones8d,
                                 rhs=ET[:, 0, 0:2, :],
                                 start=True, stop=False, perf_mode=DR)
                nc.tensor.matmul(ps_s[0:64, :], lhsT=ones8d[:, 0, :],
                                 rhs=ET[:, 0, 2, :], start=False, stop=True)
                for j in range(3):
                    nc.tensor.matmul(ps_s[64:128, :], lhsT=ones8d[:, 0, :],
                                     rhs=ET[:, 1, j, :],
                                     start=(j == 0), stop=(j == 2))
                h0 = 2 * mt
                nc.tensor.matmul(ps_o[0:64, :],
                                 lhsT=v8[:, qt:qt + 2, h0 * 64:h0 * 64 + 64],
                                 rhs=ET[:, 0, 0:2, :],
                                 start=True, stop=False, perf_mode=DR)
                nc.tensor.matmul(ps_o[0:64, :],
                                 lhsT=v8[:, qt + 2, h0 * 64:h0 * 64 + 64],
                                 rhs=ET[:, 0, 2, :], start=False, stop=True)
                h1 = 2 * mt + 1
                for j in range(3):
                    nc.tensor.matmul(ps_o[64:128, :],
                                     lhsT=v8[:, qt + j, h1 * 64:h1 * 64 + 64],
                                     rhs=ET[:, 1, j, :],
                                     start=(j == 0), stop=(j == 2))
                rec = rope_tmp.tile([128, 128], f32, tag="rec")
                nc.vector.reciprocal(out=rec, in_=ps_s)
                nc.vector.tensor_mul(out=o28[mt // 2][:, mt % 2, qt * 128:(qt + 1) * 128],
                                     in0=ps_o, in1=rec)

            x2_list = [None] * NQT
            mv2_list = [None] * NQT
            y2_list = [None] * NQT

            def outproj_chunk(qt):
                def f():
                    x2 = x2_pool.tile([128, D], bf16, tag="x2")
                    for half in range(2):
                        ps = psb.tile([128, 512], f32, tag="psqk")
                        for cn in range(2):
                            reg = ps[:, cn * 256:(cn + 1) * 256]
                            for p in range(4):
                                nc.tensor.matmul(
                                    reg,
                                    lhsT=o28[p][:, :, qt * 128:(qt + 1) * 128],
                                    rhs=wo8[:, p, :,
                                            half * 512 + cn * 256:half * 512 + (cn + 1) * 256],
                                    start=(p == 0 and cn == 0), stop=(p == 3),
                                    perf_mode=DR)
                        sl = slice(half * 512, (half + 1) * 512)
                        nc.vector.tensor_add(out=x2[:, sl], in0=ps,
                                             in1=x_tiles[2 + qt][:, sl])
                    x2_list[qt] = x2
                    mv2_list[qt] = ln_stats(x2, ln_tmp2)
                return f

            def lnfin_chunk(qt):
                def f():
                    y2 = y2_pool.tile([128, D], bf16, tag="y2")
                    ln_norm(x2_list[qt], y2, mv2_list[qt], ln_tmp2)
                    y2_list[qt] = y2
                return f

            issue_wqk(0)
            issue_wqk(1)
            issue_wqk(2)
            chunks = proj_chunks(0)
            for c in chunks:
                c()
            fill_plan = {
                (7, 1): [outproj_chunk(0), lnfin_chunk(0)],
                (7, 2): [outproj_chunk(1), lnfin_chunk(1)],
                (7, 3): [outproj_chunk(2), lnfin_chunk(2)],
            }
            ctx = None
            for mt in range(8):
                if 3 <= mt + 3 < 8:
                    issue_wqk(mt + 3)
                if mt + 1 < 8:
                    nxt = proj_chunks(mt + 1)
                for qt in range(NQT):
                    nctx = attn_front(mt, qt)
                    if ctx is not None:
                        attn_back(ctx)
                    if mt + 1 < 8:
                        nxt[qt]()
                    else:
                        for fl in fill_plan.get((mt, qt), []):
                            fl()
                    ctx = nctx
            attn_back(ctx)
            outproj_chunk(NQT - 1)()
            lnfin_chunk(3)()

        # ====== phases D: y2^T triplet interleaved with MLP gate/up ======
        H8 = []
        H8s = []
        with ExitStack() as ph:
            pst2 = ph.enter_context(tc.tile_pool(name="pst2", bufs=4, space="PSUM"))
            wgu_pool = ph.enter_context(tc.tile_pool(name="wgu", bufs=3))
            psg = ph.enter_context(tc.tile_pool(name="psg", bufs=4, space="PSUM"))
            gu_tmp = ph.enter_context(tc.tile_pool(name="gu_tmp", bufs=4))

            # y2^T split by query halves: a = tokens 0:256 (qt 0/1), b = 256:512
            y2T8a = [y2T_pool.tile([128, 2, 256], f8, name="y2T8a", tag="y2T8a")
                     for _ in range(4)]
            y2T8b = [y2T_pool.tile([128, 2, 256], f8, name="y2T8b", tag="y2T8b")
                     for _ in range(4)]
            y2T8sa = [y2T_pool.tile([128, 2, 256], f8, name="y2T8sa", tag="y2T8sa")
                      for _ in range(4)]
            y2T8sb = [y2T_pool.tile([128, 2, 256], f8, name="y2T8sb", tag="y2T8sb")
                      for _ in range(4)]
            y2Tr8a = [y2T_pool.tile([128, 2, 256], f8, name="y2Tr8a", tag="y2Tr8a")
                      for _ in range(4)] if GU_THIRD else None
            y2Tr8b = [y2T_pool.tile([128, 2, 256], f8, name="y2Tr8b", tag="y2Tr8b")
                      for _ in range(4)] if GU_THIRD else None
            gu_terms = [((y2T8a, y2T8b), 0), ((y2T8sa, y2T8sb), 1)] + \
                ([((y2Tr8a, y2Tr8b), 0)] if GU_THIRD else [])
            nterm = len(gu_terms)

            for pair in range(16):
                H8.append(hh_pool.tile([128, 2, CS], f8, name="H8", tag="hh"))
                H8s.append(hh_pool.tile([128, 2, CS], f8, name="H8s", tag="hhs"))

            def y2t_pass(half, hi_l, s_l, r_l):
                """Transpose qt pair (2*half, 2*half+1) for all 8 dtiles and
                ship the corresponding token-half fp8 triplet."""
                pts = {}
                for pair in range(4):
                    pts[pair] = pst2.tile([128, 2, 2, 128], bf16, name="pt2",
                                          tag="pst2b")
                for qi in range(2):
                    qt = half * 2 + qi
                    for pair in range(4):
                        for di in range(2):
                            dtl = pair * 2 + di
                            nc.tensor.transpose(
                                pts[pair][:, di, qi, :],
                                y2_list[qt][:, dtl * 128:(dtl + 1) * 128],
                                identb)
                for pair in range(4):
                    pt = pts[pair]
                    dst_hi = hi_l[pair][:, :, :]
                    nc.scalar.activation(out=dst_hi, in_=pt, func=AF.Identity)
                    nc.gpsimd.tensor_scalar_mul(out=s_l[pair][:, :, :],
                                                in0=dst_hi, scalar1=1.0 / 16)
                    if GU_THIRD:
                        nc.vector.tensor_tensor(out=r_l[pair][:, :, :],
                                                in0=pt, in1=dst_hi,
                                                op=OP.subtract)

            wgu_tiles = {}
            wd_tiles = {}

            def issue_wd(pair):
                w = wd_pool.tile([128, 2, 2, 1024], f8, name="wd", tag="wd")
                eng = nc.sync if pair % 2 == 0 else nc.gpsimd
                eng.dma_start(out=w, in_=wd_d[pair])
                wd_tiles[pair] = w

            def gu_mt_pass(mt, ps_pair, p, cns=(0, 1)):
                """K-pair accumulation pass of gate+up for f-block mt over the
                given column halves (cn 0 reads the a tiles, 1 the b)."""
                w = wgu_tiles[mt]
                for gi in range(2):
                    ps = ps_pair[gi]
                    for cn in cns:
                        reg = ps[:, cn * 256:(cn + 1) * 256]
                        for ti, (act, hl) in enumerate(gu_terms):
                            last = (p == 3 and ti == nterm - 1)
                            nc.tensor.matmul(
                                reg,
                                lhsT=w[:, gi, hl, p, :, :],
                                rhs=act[cn][p][:, :, :],
                                start=(p == 0 and ti == 0 and cn == 0),
                                stop=(last and not (has_bg and gi == 0)),
                                perf_mode=DR)

            def gu_mt_finish(mt, ps_pair):
                if has_bg:
                    for cn in range(2):
                        nc.tensor.matmul(
                            ps_pair[0][:, cn * 256:(cn + 1) * 256],
                            lhsT=bg_sb[:, mt * 128:(mt + 1) * 128],
                            rhs=ones_row[:, cn * 256:(cn + 1) * 256],
                            start=False, stop=True)
                U = gu_tmp.tile([128, CS], bf16, tag="U")
                nc.scalar.activation(out=U, in_=ps_pair[1], func=AF.Silu,
                                     bias=bu_sb[:, mt:mt + 1], scale=1.0)
                h8_dst = H8[mt // 2][:, mt % 2, :]
                nc.vector.tensor_mul(out=h8_dst, in0=ps_pair[0], in1=U)
                nc.gpsimd.tensor_scalar_mul(out=H8s[mt // 2][:, mt % 2, :],
                                            in0=h8_dst, scalar1=0.125)

            def new_gu_ps():
                return [psg.tile([128, CS], f32, name="psgu", tag="psgu")
                        for _ in range(2)]

            def issue_wgu(mt):
                pool = wgu0_pool if mt < 2 else wgu_pool
                w = pool.tile([128, 2, 2, 4, 2, 128], f8, name="wgu", tag="wgu")
                eng = nc.sync if mt % 2 == 0 else nc.gpsimd
                eng.dma_start(out=w, in_=wgu_d[mt])
                wgu_tiles[mt] = w

            # mt 0/1: pair passes interleaved with the y2T wave production so
            # PE stays fed while the transposes/copies stream out
            issue_wgu(0)
            issue_wgu(1)
            ps0, ps1 = new_gu_ps(), new_gu_ps()
            y2t_pass(0, y2T8a, y2T8sa, y2Tr8a)
            y2t_pass(1, y2T8b, y2T8sb, y2Tr8b)
            for p in range(4):
                gu_mt_pass(0, ps0, p, cns=(0,))
                gu_mt_pass(1, ps1, p, cns=(0,))
            for p in range(4):
                gu_mt_pass(0, ps0, p, cns=(1,))
                gu_mt_pass(1, ps1, p, cns=(1,))
            gu_mt_finish(0, ps0)
            gu_mt_finish(1, ps1)
            wgu_tiles.pop(0)
            wgu_tiles.pop(1)

            issue_wgu(2)
            for mt in range(2, 32):
                if mt + 1 < 32:
                    issue_wgu(mt + 1)
                if mt in (19, 21, 23, 25):
                    issue_wd((mt - 19) // 2)
                w = wgu_tiles[mt]
                psm = new_gu_ps()
                for p in range(4):
                    gu_mt_pass(mt, psm, p)
                gu_mt_finish(mt, psm)
                wgu_tiles.pop(mt)

        # ====== phase E: down proj (x4 weights) + residual + store ======
        with ExitStack() as ph:
            psd = ph.enter_context(tc.tile_pool(name="psd", bufs=8, space="PSUM"))
            out_pool = ph.enter_context(tc.tile_pool(name="outp", bufs=2))
            dn_tmp = ph.enter_context(tc.tile_pool(name="dn_tmp", bufs=2))

            ps_d = [psd.tile([128, 512], f32, name="psd", tag="psd") for _ in range(8)]
            dn_terms = [(H8, 0), (H8s, 1)]

            def dn_finish(tt):
                ot = out_pool.tile([128, D], f32, name="outp", tag="outp")
                for ch3 in range(2):
                    sl = slice(ch3 * 512, (ch3 + 1) * 512)
                    pd = ps_d[tt * 2 + ch3]
                    if has_bd:
                        # bias pre-scaled x4 on host to match the x4 weights
                        nc.tensor.matmul(pd[:, 0:256], lhsT=ones_row[:, 0:128],
                                         rhs=bd_sb[:, ch3 * 512:ch3 * 512 + 256],
                                         start=False, stop=True)
                        nc.tensor.matmul(pd[:, 256:512], lhsT=ones_row[:, 0:128],
                                         rhs=bd_sb[:, ch3 * 512 + 256:(ch3 + 1) * 512],
                                         start=False, stop=True)
                    tmp = dn_tmp.tile([128, 512], f32, name="dntmp", tag="dntmp")
                    nc.scalar.activation(out=tmp, in_=pd, func=AF.Identity,
                                         scale=0.25)
                    nc.vector.tensor_add(out=ot[:, sl], in0=tmp,
                                         in1=x2_list[tt][:, sl])
                eng2 = nc.sync if tt % 2 == 0 else nc.gpsimd
                eng2.dma_start(out=out_d[tt * 128:(tt + 1) * 128, :], in_=ot)

            for pair in range(16):
                if pair >= 1 and pair + 3 < 16:
                    issue_wd(pair + 3)
                w = wd_tiles.pop(pair)
                for tt in range(NQT):
                    for ti, (act, hl) in enumerate(dn_terms):
                        for cn in range(4):
                            reg = ps_d[tt * 2 + cn // 2][:, (cn % 2) * 256:
                                                         (cn % 2 + 1) * 256]
                            nc.tensor.matmul(
                                reg,
                                lhsT=act[pair][:, :, tt * 128:(tt + 1) * 128],
                                rhs=w[:, hl, :, cn * 256:(cn + 1) * 256],
                                start=(pair == 0 and ti == 0 and cn % 2 == 0),
                                stop=(pair == 15 and ti == 1 and not has_bd),
                                perf_mode=DR)
                    if pair == 15:
                        dn_finish(tt)
        mlp_scope.close()
        wqkp_scope.close()
        qkv_scope.close()
        mid_scope.close()

    nc.compile()
    return nc


def prep_inputs(x, w_qkv, w_out, g1, b1, g2, b2, w_gate, b_gate, w_up, b_up,
                w_down, b_down):
    """Host-side: fold LN params, fp8-split weights, pre-tile, build per-core
    tensors."""
    import ml_dtypes
    f32 = np.float32
    bf16 = ml_dtypes.bfloat16
    f8 = ml_dtypes.float8_e4m3

    def split8(w, s=16.0):
        hi = w.astype(f8)
        lo = ((w - hi.astype(f32)) * s).astype(f8)
        return hi, lo

    wqkv_f = (w_qkv * g1[:, None]).astype(f32)
    bqkv = (b1 @ w_qkv).astype(f32)

    def qk_tile(w):  # [D, 1024] -> [mt, p, pair, i, m] fp8 pieces
        hi, lo = split8(w)
        t = lambda a: np.ascontiguousarray(
            a.reshape(4, 2, 128, 8, 128).transpose(3, 2, 0, 1, 4))
        return t(hi), t(lo)

    qhi, qlo = qk_tile(wqkv_f[:, :D])
    khi, klo = qk_tile(wqkv_f[:, D:2 * D])
    wqk = np.ascontiguousarray(
        np.stack([qhi, qlo, khi, klo], axis=2))  # [8,128,4,4,2,128]

    def mv_tile(w):  # [D, 1024] -> [p, pair, i, n]
        return w.reshape(4, 2, 128, 1024).transpose(2, 0, 1, 3)

    vhi, vlo = split8(wqkv_f[:, 2 * D:])
    wv = np.ascontiguousarray(np.stack([mv_tile(vhi), mv_tile(vlo)], axis=1))
    wo = np.ascontiguousarray(mv_tile(w_out.astype(f32).astype(f8)))

    def gu_tile(w):  # [D, F] -> [mt, p, hi/lo, pair, i, m]
        hi, lo = split8(w)
        t = lambda a: a.reshape(4, 2, 128, 32, 128).transpose(3, 2, 0, 1, 4)
        return np.stack([t(hi), t(lo)], axis=2)  # [32,128,2,4,2,128]

    wg_f = (w_gate * g2[:, None]).astype(f32)
    wu_f = (w_up * g2[:, None]).astype(f32)
    wgu = np.ascontiguousarray(
        np.stack([gu_tile(wg_f), gu_tile(wu_f)], axis=2))  # [32,128,2,2,4,2,128]

    wd_f = w_down.astype(f32)
    wd_hi = (4.0 * wd_f).astype(f8)
    wd_lo = (32.0 * (wd_f - wd_hi.astype(f32) / 4.0)).astype(f8)
    t_wd = lambda a: a.reshape(16, 2, 128, 1024).transpose(0, 2, 1, 3)
    wd = np.ascontiguousarray(np.stack([t_wd(wd_hi), t_wd(wd_lo)], axis=2))

    bqk_pt = bqkv[:2048].reshape(16, 128).T                       # [p, t]
    bu_pt = (b_up + b2 @ w_up).astype(f32).reshape(32, 128).T
    cbf = np.ascontiguousarray(
        np.concatenate([bqk_pt, bu_pt], axis=1)).astype(f32)      # [128, 48]

    bg_row = (b_gate + b2 @ w_gate).astype(f32).reshape(1, F).astype(bf16)
    bv_row = bqkv[2048:].reshape(1, D).astype(bf16)
    bd_row = (4.0 * b_down).reshape(1, D).astype(bf16)

    # rotate-half permutation (sign folded into sin tables)
    pshuf = np.zeros((128, 128), f32)
    for m in range(128):
        base = (m // 64) * 64
        r = m % 64
        sig = base + (r + 32) % 64
        pshuf[sig, m] = 1.0
    pshuf = pshuf.astype(bf16)

    half = HD // 2
    inv_freq = 1.0 / (10000.0 ** (np.arange(half, dtype=np.float64) / half))

    def rope_tables(pos):
        t = np.maximum(pos, 0).astype(np.float64)
        freqs = np.outer(t, inv_freq)
        emb = np.concatenate([freqs, freqs], 1)
        c = np.cos(emb).T.astype(f32)
        s = np.sin(emb).T.astype(f32)
        ms = s.copy()
        ms[:32] = -ms[:32]
        return (np.ascontiguousarray(np.vstack([c, c])),
                np.ascontiguousarray(np.vstack([ms, ms])))

    common = {"wqk": wqk, "wv": wv, "wo": wo, "wgu": wgu, "wd": wd,
              "bv": bv_row, "bd": bd_row, "bg": bg_row, "cbf": cbf}

    in_maps = []
    for c in range(NCORES):
        b, chunk = c // CH, c % CH
        q0 = chunk * CS
        lo = q0 - WIN
        xh = np.zeros((HT, D), f32)
        src_lo = max(0, lo)
        xh[src_lo - lo:] = x[b, src_lo:q0 + CS]
        xh = xh.astype(bf16)
        pos_k = np.arange(lo, q0 + CS)
        cosk_a, sink_a = rope_tables(pos_k)
        # transposed multiplicative mask [r, qt, j, c]:
        #   key j_g = lo + (qt+j)*128 + r ; query i = q0 + qt*128 + c
        r_i = np.arange(128)[:, None, None, None]
        qt_i = np.arange(NQT)[None, :, None, None]
        j_i = np.arange(3)[None, None, :, None]
        c_i = np.arange(128)[None, None, None, :]
        jg = lo + (qt_i + j_i) * 128 + r_i
        gi = q0 + qt_i * 128 + c_i
        valid = (jg <= gi) & (gi - jg <= WIN) & (jg >= 0)
        maskT = valid.astype(f32).reshape(128, NQT * 3 * 128).astype(bf16)
        cbb = np.concatenate(
            [cosk_a.astype(bf16), sink_a.astype(bf16), maskT, pshuf], axis=1)
        in_maps.append(dict(common, xh=xh, cbb=np.ascontiguousarray(cbb)))
    return in_maps


_PROG = {}


def kernel(**inputs):
    from concourse.bass_utils import run_bass_kernel_spmd

    inputs = {k: np.asarray(v, dtype=np.float32) for k, v in inputs.items()}
    in_maps = prep_inputs(**inputs)
    flags = (bool(np.any(inputs["b1"] @ inputs["w_qkv"][:, 2048:])),
             bool(np.any(inputs["b_gate"] + inputs["b2"] @ inputs["w_gate"])),
             bool(np.any(inputs["b_down"])))
    if flags not in _PROG:
        _PROG[flags] = build_program(has_bv=flags[0], has_bg=flags[1],
                                     has_bd=flags[2])
    nc = _PROG[flags]
    res = run_bass_kernel_spmd(nc, in_maps, core_ids=list(range(NCORES)))
    out = np.zeros((B, S, D), np.float32)
    for c in range(NCORES):
        b, chunk = c // CH, c % CH
        out[b, chunk * CS:(chunk + 1) * CS] = res.results[c]["out"]
    return out


# revision 40
# speedup vs baseline: 1.0363x; 1.0107x over previous
"""Trainium2 Bass kernel for AdvancedTransformerEncoderBlock (fp8 DoubleRow).

Sharding: token-parallel across 8 cores (B=2 x 4 seq chunks of 512), each core
recomputes a 256-token K/V halo -> zero collectives.

Precision plan (validated vs fp32 reference, rel_err ~= 0.015):
  - qkv proj:   fp8e4 DoubleRow, weights split hi+lo(x16), activation split
                hi + hi/16 + residual  (3 passes, 4x per-pass speedup)
  - attention:  bf16 (transposed-logits flow: logits land [keys, queries] in
                PSUM; exp on Act; band mask folded into the PSUM->SBUF copy as
                a 0/1 multiply; softmax sums via ones[128,64] matmul so the
                per-query denominators arrive broadcast across partitions;
                normalize folded into the o2 copy)
  - out proj:   fp8e4 DoubleRow single-pass (o2/wo plain fp8)
  - gate/up:    like qkv (3 passes)
  - down proj:  weights split fp8(4w) + fp8(32*res), H plain fp8 + H/8 copy;
                the 4x weight prescale (keeps wd out of fp8 subnormals) is
                undone by a 0.25 scale folded into the PSUM->SBUF copy
PSUM accumulation stays fp32, residual stream stays fp32.
RoPE rotate-half runs as a PE permutation matmul.
Attention runs one query-tile ahead on logits so exp/mask latency hides under
sums/AV of the previous tile plus the interleaved projection fillers.
"""

import numpy as np

B, S, D, F, H, HD = 2, 2048, 1024, 4096, 16, 64
WIN = 256
NCORES = 8
CH = 4           # chunks per batch
CS = S // CH     # 512 tokens per chunk (queries)
HT = CS + WIN    # 768 tokens incl. halo (keys/values)
NQT = CS // 128  # 4 query tiles
EPS = 1e-5
QKV_THIRD = True   # include activation-residual pass in qkv proj
GU_THIRD = True    # include activation-residual pass in gate/up


def build_program(has_bv=False, has_bg=False, has_bd=False):
    import concourse.bass as bass
    import concourse.bacc as bacc_mod
    import concourse.tile as tile
    import concourse.mybir as mybir
    from concourse.masks import make_identity
    from contextlib import ExitStack

    dt = mybir.dt
    f32, bf16, f8 = dt.float32, dt.bfloat16, dt.float8e4
    AF = mybir.ActivationFunctionType
    OP = mybir.AluOpType
    DR = mybir.MatmulPerfMode.DoubleRow

    nc = bacc_mod.Bacc()
    Pf = lambda name, shape: nc.declare_dram_parameter(name, list(shape), f32, isOutput=False)
    Pb = lambda name, shape: nc.declare_dram_parameter(name, list(shape), bf16, isOutput=False)
    P8 = lambda name, shape: nc.declare_dram_parameter(name, list(shape), f8, isOutput=False)

    xh_d = Pb("xh", (HT, D))
    wqk_d = P8("wqk", (8, 128, 4, 4, 2, 128))   # [mt][p][qhi,qlo,khi,klo][pair][i][m]
    wv_d = P8("wv", (128, 2, 4, 2, 1024))       # [p][hi/lo][pair][i][n]
    wo_d = P8("wo", (128, 4, 2, 1024))          # [p][pair][i][n]
    wgu_d = P8("wgu", (32, 128, 2, 2, 4, 2, 128))  # [mt][p][g/u][hi/lo][pair][i][m]
    wd_d = P8("wd", (16, 128, 2, 2, 1024))      # [pair][p][hi/lo][i][n]
    bv_d = Pb("bv", (1, D))
    bd_d = Pb("bd", (1, D))
    bg_d = Pb("bg", (1, F))
    cbf_d = Pf("cbf", (128, 48))                # bqk [:,0:16], bu [:,16:48]
    cbb_d = Pb("cbb", (128, 3200))
    out_d = nc.declare_dram_parameter("out", [CS, D], f32, isOutput=True)

    with tile.TileContext(nc) as tc, ExitStack() as top:
        const = top.enter_context(tc.tile_pool(name="const", bufs=1))

        # x tiles first: their DMAs head the queue so LN/transposes start early
        x_pool = top.enter_context(tc.tile_pool(name="x", bufs=6))
        x_tiles = []
        for tt in range(6):
            xt = x_pool.tile([128, D], bf16, tag="xt")
            eng = nc.sync if tt % 2 == 0 else nc.gpsimd
            if tt == 0:
                eng.dma_start(out=xt[:, 0:512], in_=xh_d[0:128, 0:512])
                eng.dma_start(out=xt[:, 512:1024], in_=xh_d[0:128, 512:1024])
            else:
                eng.dma_start(out=xt, in_=xh_d[tt * 128:(tt + 1) * 128, :])
            x_tiles.append(xt)

        # ---- constants ----
        cbf = const.tile([128, 48], f32, tag="cbf")
        nc.sync.dma_start(out=cbf, in_=cbf_d[:, :])
        cbb = const.tile([128, 3200], bf16, tag="cbb")
        nc.gpsimd.dma_start(out=cbb, in_=cbb_d[:, :])
        bqk_sb = cbf[:, 0:16]
        bu_sb = cbf[:, 16:48]
        cosk = cbb[:, 0:768]
        msink = cbb[:, 768:1536]
        cosq = cbb[:, WIN:768]
        msinq = cbb[:, 768 + WIN:1536]
        masks = [cbb[:, 1536 + qt * 384:1536 + (qt + 1) * 384] for qt in range(NQT)]
        pshuf = cbb[:, 3072:3200]

        identb = const.tile([128, 128], bf16, tag="identb")
        make_identity(nc, identb)
        ones64 = const.tile([128, 64], bf16, tag="ones64")
        nc.vector.memset(ones64, 1.0)
        ones_row = const.tile([1, 512], bf16, tag="ones_row")
        nc.vector.memset(ones_row, 1.0)
        eps_t = const.tile([128, 1], f32, tag="eps")
        nc.vector.memset(eps_t, EPS)
        esh_t = const.tile([128, 1], f32, tag="esh")
        nc.vector.memset(esh_t, -3.0)
        if has_bv:
            bv_sb = const.tile([1, D], bf16, tag="bv")
            nc.sync.dma_start(out=bv_sb, in_=bv_d[:, :])
        if has_bd:
            bd_sb = const.tile([1, D], bf16, tag="bd")
            nc.sync.dma_start(out=bd_sb, in_=bd_d[:, :])
        if has_bg:
            bg_sb = const.tile([1, F], bf16, tag="bg")
            nc.sync.dma_start(out=bg_sb, in_=bg_d[:, :])

        # ---- persistent activation pools ----
        x2_pool = top.enter_context(tc.tile_pool(name="x2", bufs=4))
        y2T_pool = top.enter_context(tc.tile_pool(name="y2T", bufs=4))
        o2_pool = top.enter_context(tc.tile_pool(name="o2", bufs=4))

        def ln_stats(src, tmp_pool):
            stats = tmp_pool.tile([128, 2, 6], f32, tag="lnstats")
            mv = tmp_pool.tile([128, 2], f32, tag="lnmv")
            for sg in range(2):
                nc.vector.bn_stats(out=stats[:, sg, :], in_=src[:, sg * 512:(sg + 1) * 512])
            nc.vector.bn_aggr(out=mv, in_=stats)
            return mv

        def ln_norm(src, dst, mv, tmp_pool):
            rs = tmp_pool.tile([128, 1], f32, tag="lnrs")
            nc.scalar.activation(out=rs, in_=mv[:, 1:2], func=AF.Sqrt,
                                 bias=eps_t, scale=1.0)
            nc.vector.reciprocal(out=rs, in_=rs)
            nb = tmp_pool.tile([128, 1], f32, tag="lnnb")
            nc.vector.tensor_scalar(out=nb, in0=mv[:, 0:1], scalar1=rs,
                                    scalar2=-1.0, op0=OP.mult, op1=OP.mult)
            nc.scalar.activation(out=dst, in_=src, func=AF.Identity,
                                 bias=nb, scale=rs)

        def layernorm(src, dst, tmp_pool):
            ln_norm(src, dst, ln_stats(src, tmp_pool), tmp_pool)

        # mid-lived: x2-LN workspace + y2 (read by the late y2T transposes)
        mid_scope = ExitStack()
        ln_tmp2 = mid_scope.enter_context(tc.tile_pool(name="ln_tmp2", bufs=3))
        y2_pool = mid_scope.enter_context(tc.tile_pool(name="y2", bufs=4))

        qkv_scope = ExitStack()
        yT_pool = qkv_scope.enter_context(tc.tile_pool(name="yT", bufs=4))
        wo_pool = qkv_scope.enter_context(tc.tile_pool(name="wo", bufs=1))
        wo8 = wo_pool.tile([128, 4, 2, 1024], f8, tag="wo8")
        qT_pool = qkv_scope.enter_context(tc.tile_pool(name="qT", bufs=3))
        kT_pool = qkv_scope.enter_context(tc.tile_pool(name="kT", bufs=3))
        vb_pool = qkv_scope.enter_context(tc.tile_pool(name="vb", bufs=6))


        # y^T pair tiles split by token halves so consumers start after the
        # first three LN outputs: a = tokens 0:384, b = 384:768
        HH = HT // 2
        yT8a = [yT_pool.tile([128, 2, HH], f8, name="yT8a", tag="yT8a") for _ in range(4)]
        yT8b = [yT_pool.tile([128, 2, HH], f8, name="yT8b", tag="yT8b") for _ in range(4)]
        yT8sa = [yT_pool.tile([128, 2, HH], f8, name="yT8sa", tag="yT8sa") for _ in range(4)]
        yT8sb = [yT_pool.tile([128, 2, HH], f8, name="yT8sb", tag="yT8sb") for _ in range(4)]
        yTr8a = [yT_pool.tile([128, 2, HH], f8, name="yTr8a", tag="yTr8a")
                 for _ in range(4)] if QKV_THIRD else None
        yTr8b = [yT_pool.tile([128, 2, HH], f8, name="yTr8b", tag="yTr8b")
                 for _ in range(4)] if QKV_THIRD else None

        # =========== phase A: LN1 -> y -> y^T fp8 triplet ===========
        with ExitStack() as ph:
            ln_tmp = ph.enter_context(tc.tile_pool(name="ln_tmp", bufs=6))
            y_pool = ph.enter_context(tc.tile_pool(name="y", bufs=6))
            pst = ph.enter_context(tc.tile_pool(name="pst", bufs=8, space="PSUM"))

            ys = []
            for tt in range(6):
                y = y_pool.tile([128, D], bf16, tag="y")
                layernorm(x_tiles[tt], y, ln_tmp)
                ys.append(y)
            # tt-outer with half-granular copies: the a-half (tokens 0:384)
            # ships as soon as the first three LN outputs exist
            pts = [pst.tile([128, 6, 128], bf16, name="pt", tag="pst")
                   for _ in range(8)]
            for half, (hi_l, s_l, r_l) in enumerate(
                    [(yT8a, yT8sa, yTr8a), (yT8b, yT8sb, yTr8b)]):
                for tt in range(half * 3, half * 3 + 3):
                    for dtl in range(8):
                        nc.tensor.transpose(pts[dtl][:, tt, :],
                                            ys[tt][:, dtl * 128:(dtl + 1) * 128],
                                            identb)
                for dtl in range(8):
                    pt = pts[dtl][:, half * 3:half * 3 + 3, :]
                    dst_hi = hi_l[dtl // 2][:, dtl % 2, :]
                    nc.scalar.activation(out=dst_hi, in_=pt, func=AF.Identity)
                    nc.gpsimd.tensor_scalar_mul(out=s_l[dtl // 2][:, dtl % 2, :],
                                                in0=dst_hi, scalar1=1.0 / 16)
                    if QKV_THIRD:
                        nc.vector.tensor_tensor(out=r_l[dtl // 2][:, dtl % 2, :],
                                                in0=pt, in1=dst_hi, op=OP.subtract)

        # late-lived pools opened after phase A so their space reuses the LN
        # workspace; DMAs for v/out-proj weights head the queue here
        wqkp_scope = ExitStack()
        wqk_pool = wqkp_scope.enter_context(tc.tile_pool(name="wqkp", bufs=3))
        mlp_scope = ExitStack()
        hh_pool = mlp_scope.enter_context(tc.tile_pool(name="hh", bufs=16))
        wd_pool = mlp_scope.enter_context(tc.tile_pool(name="wd", bufs=3))
        wgu0_pool = mlp_scope.enter_context(tc.tile_pool(name="wgu0", bufs=2))
        vw_scope = ExitStack()
        wv_pool = vw_scope.enter_context(tc.tile_pool(name="wv", bufs=1))
        wv8 = wv_pool.tile([128, 2, 4, 2, 1024], f8, tag="wv8")
        for p_ in range(4):
            eng = nc.sync if p_ % 2 == 0 else nc.gpsimd
            eng.dma_start(out=wv8[:, :, p_, :, :], in_=wv_d[:, :, p_, :, :])
        nc.gpsimd.dma_start(out=wo8, in_=wo_d[:, :, :, :])

        qkv_terms = [(yT8a, yT8b, 0), (yT8sa, yT8sb, 1)] + \
            ([(yTr8a, yTr8b, 0)] if QKV_THIRD else [])

        def ytok(term, lo, hi):
            """AP for token range [lo, hi) of a qkv term (within one half)."""
            a_l, b_l, _ = term
            if hi <= HH:
                return lambda p: a_l[p][:, :, lo:hi]
            assert lo >= HH
            return lambda p: b_l[p][:, :, lo - HH:hi - HH]

        # =========== phase B: v projection (fp8 DR, pair-outer) ===========
        with ExitStack() as ph:
            psv = ph.enter_context(tc.tile_pool(name="psv", bufs=6, space="PSUM"))
            v8 = vb_pool.tile([128, 6, D], f8, tag="vbf")
            nterm = len(qkv_terms)
            for chv in range(2):
                pv = [psv.tile([128, 512], f32, name="psv", tag="psv") for _ in range(6)]
                for ti, term in enumerate(qkv_terms):
                    hl = term[2]
                    for p in range(4):
                        for tt in range(6):
                            lsrc = ytok(term, tt * 128, (tt + 1) * 128)(p)
                            for cn in range(2):
                                reg = pv[tt][:, cn * 256:(cn + 1) * 256]
                                last = (p == 3 and ti == nterm - 1)
                                nc.tensor.matmul(
                                    reg,
                                    lhsT=lsrc,
                                    rhs=wv8[:, hl, p, :,
                                            chv * 512 + cn * 256:chv * 512 + (cn + 1) * 256],
                                    start=(ti == 0 and p == 0 and cn == 0),
                                    stop=(last and not has_bv), perf_mode=DR)
                if has_bv:
                    for tt in range(6):
                        for cn in range(2):
                            nc.tensor.matmul(pv[tt][:, cn * 256:(cn + 1) * 256],
                                             lhsT=ones_row[:, 0:128],
                                             rhs=bv_sb[:, chv * 512 + cn * 256:
                                                       chv * 512 + (cn + 1) * 256],
                                             start=False, stop=True)
                for tt in range(6):
                    sl = slice(chv * 512, (chv + 1) * 512)
                    if tt % 2 == 0:
                        nc.scalar.copy(out=v8[:, tt, sl], in_=pv[tt])
                    else:
                        nc.vector.tensor_copy(out=v8[:, tt, sl], in_=pv[tt])
        vw_scope.close()

        # ==== phase C: q/k proj + RoPE pipelined with attention (flow B) ====
        qT, kT = [], []
        with ExitStack() as ph:
            psb = ph.enter_context(tc.tile_pool(name="psb", bufs=3, space="PSUM"))
            rope_tmp = ph.enter_context(tc.tile_pool(name="rope_tmp", bufs=2))
            at = ph.enter_context(tc.tile_pool(name="at", bufs=3))
            atb = ph.enter_context(tc.tile_pool(name="atb", bufs=2))
            psl = ph.enter_context(tc.tile_pool(name="psl", bufs=2, space="PSUM"))
            pss = ph.enter_context(tc.tile_pool(name="pss", bufs=1, space="PSUM"))
            pso = ph.enter_context(tc.tile_pool(name="pso", bufs=2, space="PSUM"))

            o28 = [o2_pool.tile([128, 2, CS], f8, name="o28", tag="o28")
                   for _ in range(4)]

            wqk_tiles = {}

            def issue_wqk(mt):
                w = wqk_pool.tile([128, 4, 4, 2, 128], f8, name="wqk", tag="wqk")
                nc.sync.dma_start(out=w, in_=wqk_d[mt])
                wqk_tiles[mt] = w

            def proj_chunks(mt):
                w = wqk_tiles.pop(mt)
                qt_t = qT_pool.tile([128, CS], bf16, tag="qT")
                kt_t = kT_pool.tile([128, HT], bf16, tag="kT")
                st = {}

                def emit_proj(ps, wbase, tok_lo, tok_hi):
                    cuts = sorted({tok_lo, tok_hi}
                                  | {c for c in (HH, 256, 640) if tok_lo < c < tok_hi})
                    chunks = list(zip(cuts[:-1], cuts[1:]))
                    for cn, (c0, c1) in enumerate(chunks):
                        reg = ps[:, c0 - tok_lo:c1 - tok_lo]
                        for ti, term in enumerate(qkv_terms):
                            hl = term[2]
                            for p in range(4):
                                nc.tensor.matmul(
                                    reg,
                                    lhsT=w[:, wbase + hl, p, :, :],
                                    rhs=ytok(term, c0, c1)(p),
                                    start=(ti == 0 and p == 0 and cn == 0),
                                    stop=(ti == len(qkv_terms) - 1 and p == 3),
                                    perf_mode=DR)

                def c0():  # q projection
                    ps = psb.tile([128, CS], f32, tag="psqk")
                    emit_proj(ps, 0, WIN, HT)
                    qb = rope_tmp.tile([128, CS], bf16, tag="ropesrc")
                    nc.scalar.activation(out=qb, in_=ps, func=AF.Identity,
                                         bias=bqk_sb[:, mt:mt + 1], scale=1.0)
                    st["qb"] = qb

                def c1():  # q rope
                    qb = st["qb"]
                    pr = psb.tile([128, 512], f32, tag="psqk")
                    nc.tensor.matmul(pr, lhsT=pshuf, rhs=qb, start=True, stop=True)
                    u = rope_tmp.tile([128, HT], bf16, tag="ropeu")
                    nc.vector.tensor_mul(out=u[:, :CS], in0=qb, in1=cosq)
                    t1 = rope_tmp.tile([128, 512], bf16, tag="ropet")
                    nc.vector.tensor_mul(out=t1, in0=pr, in1=msinq)
                    nc.vector.tensor_add(out=qt_t, in0=u[:, :CS], in1=t1)

                def c2():  # k projection half 0
                    kb = rope_tmp.tile([128, HT], bf16, tag="ropesrck")
                    st["kb"] = kb
                    ps = psb.tile([128, 384], f32, tag="psqk")
                    emit_proj(ps, 2, 0, 384)
                    nc.scalar.activation(out=kb[:, 0:384], in_=ps, func=AF.Identity,
                                         bias=bqk_sb[:, 8 + mt:9 + mt], scale=1.0)

                def c3():  # k projection half 1 + k rope
                    kb = st["kb"]
                    ps = psb.tile([128, 384], f32, tag="psqk")
                    emit_proj(ps, 2, 384, HT)
                    nc.scalar.activation(out=kb[:, 384:768], in_=ps, func=AF.Identity,
                                         bias=bqk_sb[:, 8 + mt:9 + mt], scale=1.0)
                    u = rope_tmp.tile([128, HT], bf16, tag="ropeu")
                    nc.vector.tensor_mul(out=u, in0=kb, in1=cosk)
                    for c in range(2):
                        w_ = 512 if c == 0 else 256
                        sl_ = slice(c * 512, c * 512 + w_)
                        pr = psb.tile([128, 512], f32, tag="psqk")
                        nc.tensor.matmul(pr[:, :w_], lhsT=pshuf, rhs=kb[:, sl_],
                                         start=True, stop=True)
                        t1 = rope_tmp.tile([128, 512], bf16, tag="ropet")
                        nc.vector.tensor_mul(out=t1[:, :w_], in0=pr[:, :w_],
                                             in1=msink[:, sl_])
                        nc.vector.tensor_add(out=kt_t[:, sl_], in0=u[:, sl_],
                                             in1=t1[:, :w_])

                qT.append(qt_t)
                kT.append(kt_t)
                return [c0, c1, c2, c3]

            def attn_front(mt, qt):
                """logits (PE) + exp (Act) + mask-mult (DVE) -> ET."""
                ps_l2 = []
                for hh in range(2):
                    hr = hh * 64
                    ps_l = psl.tile([128, 384], f32, tag="psl")
                    for j in range(3):
                        nc.tensor.matmul(
                            ps_l[:, j * 128:(j + 1) * 128],
                            lhsT=kT[mt][hr:hr + 64, (qt + j) * 128:(qt + j + 1) * 128],
                            rhs=qT[mt][hr:hr + 64, qt * 128:(qt + 1) * 128],
                            start=(j == 0), stop=(j == 2))
                    ps_l2.append(ps_l)
                Eb = atb.tile([128, 2, 3, 128], bf16, tag="Eb")
                for hh in range(2):
                    nc.scalar.activation(out=Eb[:, hh, :, :], in_=ps_l2[hh],
                                         func=AF.Exp, scale=float(HD) ** -0.5)
                ET = at.tile([128, 2, 3, 128], bf16, tag="ET")
                for hh in range(2):
                    nc.vector.tensor_mul(out=ET[:, hh, :, :], in0=Eb[:, hh, :, :],
                                         in1=masks[qt])
                return (mt, qt, ET)

            def attn_back(ctx):
                """sums + AV (PE), then normalize into o28 (DVE)."""
                mt, qt, ET = ctx
                ps_s = pss.tile([128, 128], f32, tag="pss")
                ps_o = pso.tile([128, 128], f32, tag="pso")
                for hh in range(2):
                    hr = hh * 64
                    for j in range(3):
                        nc.tensor.matmul(ps_s[hr:hr + 64, :], lhsT=ones64,
                                         rhs=ET[:, hh, j, :],
                                         start=(j == 0), stop=(j == 2))
                for hh in range(2):
                    hr = hh * 64
                    h = 2 * mt + hh
                    for j in range(3):
                        nc.tensor.matmul(ps_o[hr:hr + 64, :],
                                         lhsT=v8[:, qt + j, h * 64:h * 64 + 64],
                                         rhs=ET[:, hh, j, :],
                                         start=(j == 0), stop=(j == 2))
                rec = rope_tmp.tile([128, 128], f32, tag="rec")
                nc.vector.reciprocal(out=rec, in_=ps_s)
                nc.vector.tensor_mul(out=o28[mt // 2][:, mt % 2, qt * 128:(qt + 1) * 128],
                                     in0=ps_o, in1=rec)

            x2_list = [None] * NQT
            mv2_list = [None] * NQT
            y2_list = [None] * NQT

            def outproj_chunk(qt):
                def f():
                    x2 = x2_pool.tile([128, D], bf16, tag="x2")
                    for half in range(2):
                        ps = psb.tile([128, 512], f32, tag="psqk")
                        for cn in range(2):
                            reg = ps[:, cn * 256:(cn + 1) * 256]
                            for p in range(4):
                                nc.tensor.matmul(
                                    reg,
                                    lhsT=o28[p][:, :, qt * 128:(qt + 1) * 128],
                                    rhs=wo8[:, p, :,
                                            half * 512 + cn * 256:half * 512 + (cn + 1) * 256],
                                    start=(p == 0 and cn == 0), stop=(p == 3),
                                    perf_mode=DR)
                        sl = slice(half * 512, (half + 1) * 512)
                        nc.vector.tensor_add(out=x2[:, sl], in0=ps,
                                             in1=x_tiles[2 + qt][:, sl])
                    x2_list[qt] = x2
                    mv2_list[qt] = ln_stats(x2, ln_tmp2)
                return f

            def lnfin_chunk(qt):
                def f():
                    y2 = y2_pool.tile([128, D], bf16, tag="y2")
                    ln_norm(x2_list[qt], y2, mv2_list[qt], ln_tmp2)
                    y2_list[qt] = y2
                return f

            issue_wqk(0)
            issue_wqk(1)
            issue_wqk(2)
            chunks = proj_chunks(0)
            for c in chunks:
                c()
            fill_plan = {
                (7, 1): [outproj_chunk(0), lnfin_chunk(0)],
                (7, 2): [outproj_chunk(1), lnfin_chunk(1)],
                (7, 3): [outproj_chunk(2), lnfin_chunk(2)],
            }
            ctx = None
            for mt in range(8):
                if 3 <= mt + 3 < 8:
                    issue_wqk(mt + 3)
                if mt + 1 < 8:
                    nxt = proj_chunks(mt + 1)
                for qt in range(NQT):
                    nctx = attn_front(mt, qt)
                    if ctx is not None:
                        attn_back(ctx)
                    if mt + 1 < 8:
                        nxt[qt]()
                    else:
                        for fl in fill_plan.get((mt, qt), []):
                            fl()
                    ctx = nctx
            attn_back(ctx)
            outproj_chunk(NQT - 1)()
            lnfin_chunk(3)()

        # ====== phases D: y2^T triplet interleaved with MLP gate/up ======
        H8 = []
        H8s = []
        with ExitStack() as ph:
            pst2 = ph.enter_context(tc.tile_pool(name="pst2", bufs=4, space="PSUM"))
            wgu_pool = ph.enter_context(tc.tile_pool(name="wgu", bufs=3))
            psg = ph.enter_context(tc.tile_pool(name="psg", bufs=4, space="PSUM"))
            gu_tmp = ph.enter_context(tc.tile_pool(name="gu_tmp", bufs=4))

            # y2^T split by query halves: a = tokens 0:256 (qt 0/1), b = 256:512
            y2T8a = [y2T_pool.tile([128, 2, 256], f8, name="y2T8a", tag="y2T8a")
                     for _ in range(4)]
            y2T8b = [y2T_pool.tile([128, 2, 256], f8, name="y2T8b", tag="y2T8b")
                     for _ in range(4)]
            y2T8sa = [y2T_pool.tile([128, 2, 256], f8, name="y2T8sa", tag="y2T8sa")
                      for _ in range(4)]
            y2T8sb = [y2T_pool.tile([128, 2, 256], f8, name="y2T8sb", tag="y2T8sb")
                      for _ in range(4)]
            y2Tr8a = [y2T_pool.tile([128, 2, 256], f8, name="y2Tr8a", tag="y2Tr8a")
                      for _ in range(4)] if GU_THIRD else None
            y2Tr8b = [y2T_pool.tile([128, 2, 256], f8, name="y2Tr8b", tag="y2Tr8b")
                      for _ in range(4)] if GU_THIRD else None
            gu_terms = [((y2T8a, y2T8b), 0), ((y2T8sa, y2T8sb), 1)] + \
                ([((y2Tr8a, y2Tr8b), 0)] if GU_THIRD else [])
            nterm = len(gu_terms)

            for pair in range(16):
                H8.append(hh_pool.tile([128, 2, CS], f8, name="H8", tag="hh"))
                H8s.append(hh_pool.tile([128, 2, CS], f8, name="H8s", tag="hhs"))

            def y2t_pass(half, hi_l, s_l, r_l):
                """Transpose qt pair (2*half, 2*half+1) for all 8 dtiles and
                ship the corresponding token-half fp8 triplet."""
                pts = {}
                for pair in range(4):
                    pts[pair] = pst2.tile([128, 2, 2, 128], bf16, name="pt2",
                                          tag="pst2b")
                for qi in range(2):
                    qt = half * 2 + qi
                    for pair in range(4):
                        for di in range(2):
                            dtl = pair * 2 + di
                            nc.tensor.transpose(
                                pts[pair][:, di, qi, :],
                                y2_list[qt][:, dtl * 128:(dtl + 1) * 128],
                                identb)
                for pair in range(4):
                    pt = pts[pair]
                    dst_hi = hi_l[pair][:, :, :]
                    nc.scalar.activation(out=dst_hi, in_=pt, func=AF.Identity)
                    nc.gpsimd.tensor_scalar_mul(out=s_l[pair][:, :, :],
                                                in0=dst_hi, scalar1=1.0 / 16)
                    if GU_THIRD:
                        nc.vector.tensor_tensor(out=r_l[pair][:, :, :],
                                                in0=pt, in1=dst_hi,
                                                op=OP.subtract)

            wgu_tiles = {}
            wd_tiles = {}

            def issue_wd(pair):
                w = wd_pool.tile([128, 2, 2, 1024], f8, name="wd", tag="wd")
                eng = nc.sync if pair % 2 == 0 else nc.gpsimd
                eng.dma_start(out=w, in_=wd_d[pair])
                wd_tiles[pair] = w

            def gu_mt_pass(mt, ps_pair, p, cns=(0, 1)):
                """K-pair accumulation pass of gate+up for f-block mt over the
                given column halves (cn 0 reads the a tiles, 1 the b)."""
                w = wgu_tiles[mt]
                for gi in range(2):
                    ps = ps_pair[gi]
                    for cn in cns:
                        reg = ps[:, cn * 256:(cn + 1) * 256]
                        for ti, (act, hl) in enumerate(gu_terms):
                            last = (p == 3 and ti == nterm - 1)
                            nc.tensor.matmul(
                                reg,
                                lhsT=w[:, gi, hl, p, :, :],
                                rhs=act[cn][p][:, :, :],
                                start=(p == 0 and ti == 0 and cn == 0),
                                stop=(last and not (has_bg and gi == 0)),
                                perf_mode=DR)

            def gu_mt_finish(mt, ps_pair):
                if has_bg:
                    for cn in range(2):
                        nc.tensor.matmul(
                            ps_pair[0][:, cn * 256:(cn + 1) * 256],
                            lhsT=bg_sb[:, mt * 128:(mt + 1) * 128],
                            rhs=ones_row[:, cn * 256:(cn + 1) * 256],
                            start=False, stop=True)
                U = gu_tmp.tile([128, CS], bf16, tag="U")
                nc.scalar.activation(out=U, in_=ps_pair[1], func=AF.Silu,
                                     bias=bu_sb[:, mt:mt + 1], scale=1.0)
                h8_dst = H8[mt // 2][:, mt % 2, :]
                nc.vector.tensor_mul(out=h8_dst, in0=ps_pair[0], in1=U)
                nc.gpsimd.tensor_scalar_mul(out=H8s[mt // 2][:, mt % 2, :],
                                            in0=h8_dst, scalar1=0.125)

            def new_gu_ps():
                return [psg.tile([128, CS], f32, name="psgu", tag="psgu")
                        for _ in range(2)]

            def issue_wgu(mt):
                pool = wgu0_pool if mt < 2 else wgu_pool
                w = pool.tile([128, 2, 2, 4, 2, 128], f8, name="wgu", tag="wgu")
                eng = nc.sync if mt % 2 == 0 else nc.gpsimd
                eng.dma_start(out=w, in_=wgu_d[mt])
                wgu_tiles[mt] = w

            # mt 0/1: pair passes interleaved with the y2T wave production so
            # PE stays fed while the transposes/copies stream out
            issue_wgu(0)
            issue_wgu(1)
            ps0, ps1 = new_gu_ps(), new_gu_ps()
            y2t_pass(0, y2T8a, y2T8sa, y2Tr8a)
            y2t_pass(1, y2T8b, y2T8sb, y2Tr8b)
            for p in range(4):
                gu_mt_pass(0, ps0, p, cns=(0,))
                gu_mt_pass(1, ps1, p, cns=(0,))
            for p in range(4):
                gu_mt_pass(0, ps0, p, cns=(1,))
                gu_mt_pass(1, ps1, p, cns=(1,))
            gu_mt_finish(0, ps0)
            gu_mt_finish(1, ps1)
            wgu_tiles.pop(0)
            wgu_tiles.pop(1)

            issue_wgu(2)
            for mt in range(2, 32):
                if mt + 1 < 32:
                    issue_wgu(mt + 1)
                if mt in (21, 23, 25):
                    issue_wd((mt - 21) // 2)
                w = wgu_tiles[mt]
                psm = new_gu_ps()
                for p in range(4):
                    gu_mt_pass(mt, psm, p)
                gu_mt_finish(mt, psm)
                wgu_tiles.pop(mt)

        # ====== phase E: down proj (x4 weights) + residual + store ======
        with ExitStack() as ph:
            psd = ph.enter_context(tc.tile_pool(name="psd", bufs=8, space="PSUM"))
            out_pool = ph.enter_context(tc.tile_pool(name="outp", bufs=2))
            dn_tmp = ph.enter_context(tc.tile_pool(name="dn_tmp", bufs=2))

            ps_d = [psd.tile([128, 512], f32, name="psd", tag="psd") for _ in range(8)]
            dn_terms = [(H8, 0), (H8s, 1)]

            def dn_finish(tt):
                ot = out_pool.tile([128, D], f32, name="outp", tag="outp")
                for ch3 in range(2):
                    sl = slice(ch3 * 512, (ch3 + 1) * 512)
                    pd = ps_d[tt * 2 + ch3]
                    if has_bd:
                        # bias pre-scaled x4 on host to match the x4 weights
                        nc.tensor.matmul(pd[:, 0:256], lhsT=ones_row[:, 0:128],
                                         rhs=bd_sb[:, ch3 * 512:ch3 * 512 + 256],
                                         start=False, stop=True)
                        nc.tensor.matmul(pd[:, 256:512], lhsT=ones_row[:, 0:128],
                                         rhs=bd_sb[:, ch3 * 512 + 256:(ch3 + 1) * 512],
                                         start=False, stop=True)
                    tmp = dn_tmp.tile([128, 512], f32, name="dntmp", tag="dntmp")
                    nc.scalar.activation(out=tmp, in_=pd, func=AF.Identity,
                                         scale=0.25)
                    nc.vector.tensor_add(out=ot[:, sl], in0=tmp,
                                         in1=x2_list[tt][:, sl])
                eng2 = nc.sync if tt % 2 == 0 else nc.gpsimd
                eng2.dma_start(out=out_d[tt * 128:(tt + 1) * 128, :], in_=ot)

            for pair in range(16):
                if pair >= 1 and pair + 2 < 16:
                    issue_wd(pair + 2)
                w = wd_tiles.pop(pair)
                for tt in range(NQT):
                    for ti, (act, hl) in enumerate(dn_terms):
                        for cn in range(4):
                            reg = ps_d[tt * 2 + cn // 2][:, (cn % 2) * 256:
                                                         (cn % 2 + 1) * 256]
                            nc.tensor.matmul(
                                reg,
                                lhsT=act[pair][:, :, tt * 128:(tt + 1) * 128],
                                rhs=w[:, hl, :, cn * 256:(cn + 1) * 256],
                                start=(pair == 0 and ti == 0 and cn % 2 == 0),
                                stop=(pair == 15 and ti == 1 and not has_bd),
                                perf_mode=DR)
                    if pair == 15:
                        dn_finish(tt)
        mlp_scope.close()
        wqkp_scope.close()
        qkv_scope.close()
        mid_scope.close()

    nc.compile()
    return nc


def prep_inputs(x, w_qkv, w_out, g1, b1, g2, b2, w_gate, b_gate, w_up, b_up,
                w_down, b_down):
    """Host-side: fold LN params, fp8-split weights, pre-tile, build per-core
    tensors."""
    import ml_dtypes
    f32 = np.float32
    bf16 = ml_dtypes.bfloat16
    f8 = ml_dtypes.float8_e4m3

    def split8(w, s=16.0):
        hi = w.astype(f8)
        lo = ((w - hi.astype(f32)) * s).astype(f8)
        return hi, lo

    wqkv_f = (w_qkv * g1[:, None]).astype(f32)
    bqkv = (b1 @ w_qkv).astype(f32)

    def qk_tile(w):  # [D, 1024] -> [mt, p, pair, i, m] fp8 pieces
        hi, lo = split8(w)
        t = lambda a: np.ascontiguousarray(
            a.reshape(4, 2, 128, 8, 128).transpose(3, 2, 0, 1, 4))
        return t(hi), t(lo)

    qhi, qlo = qk_tile(wqkv_f[:, :D])
    khi, klo = qk_tile(wqkv_f[:, D:2 * D])
    wqk = np.ascontiguousarray(
        np.stack([qhi, qlo, khi, klo], axis=2))  # [8,128,4,4,2,128]

    def mv_tile(w):  # [D, 1024] -> [p, pair, i, n]
        return w.reshape(4, 2, 128, 1024).transpose(2, 0, 1, 3)

    vhi, vlo = split8(wqkv_f[:, 2 * D:])
    wv = np.ascontiguousarray(np.stack([mv_tile(vhi), mv_tile(vlo)], axis=1))
    wo = np.ascontiguousarray(mv_tile(w_out.astype(f32).astype(f8)))

    def gu_tile(w):  # [D, F] -> [mt, p, hi/lo, pair, i, m]
        hi, lo = split8(w)
        t = lambda a: a.reshape(4, 2, 128, 32, 128).transpose(3, 2, 0, 1, 4)
        return np.stack([t(hi), t(lo)], axis=2)  # [32,128,2,4,2,128]

    wg_f = (w_gate * g2[:, None]).astype(f32)
    wu_f = (w_up * g2[:, None]).astype(f32)
    wgu = np.ascontiguousarray(
        np.stack([gu_tile(wg_f), gu_tile(wu_f)], axis=2))  # [32,128,2,2,4,2,128]

    wd_f = w_down.astype(f32)
    wd_hi = (4.0 * wd_f).astype(f8)
    wd_lo = (32.0 * (wd_f - wd_hi.astype(f32) / 4.0)).astype(f8)
    t_wd = lambda a: a.reshape(16, 2, 128, 1024).transpose(0, 2, 1, 3)
    wd = np.ascontiguousarray(np.stack([t_wd(wd_hi), t_wd(wd_lo)], axis=2))

    bqk_pt = bqkv[:2048].reshape(16, 128).T                       # [p, t]
    bu_pt = (b_up + b2 @ w_up).astype(f32).reshape(32, 128).T
    cbf = np.ascontiguousarray(
        np.concatenate([bqk_pt, bu_pt], axis=1)).astype(f32)      # [128, 48]

    bg_row = (b_gate + b2 @ w_gate).astype(f32).reshape(1, F).astype(bf16)
    bv_row = bqkv[2048:].reshape(1, D).astype(bf16)
    bd_row = (4.0 * b_down).reshape(1, D).astype(bf16)

    # rotate-half permutation (sign folded into sin tables)
    pshuf = np.zeros((128, 128), f32)
    for m in range(128):
        base = (m // 64) * 64
        r = m % 64
        sig = base + (r + 32) % 64
        pshuf[sig, m] = 1.0
    pshuf = pshuf.astype(bf16)

    half = HD // 2
    inv_freq = 1.0 / (10000.0 ** (np.arange(half, dtype=np.float64) / half))

    def rope_tables(pos):
        t = np.maximum(pos, 0).astype(np.float64)
        freqs = np.outer(t, inv_freq)
        emb = np.concatenate([freqs, freqs], 1)
        c = np.cos(emb).T.astype(f32)
        s = np.sin(emb).T.astype(f32)
        ms = s.copy()
        ms[:32] = -ms[:32]
        return (np.ascontiguousarray(np.vstack([c, c])),
                np.ascontiguousarray(np.vstack([ms, ms])))

    common = {"wqk": wqk, "wv": wv, "wo": wo, "wgu": wgu, "wd": wd,
              "bv": bv_row, "bd": bd_row, "bg": bg_row, "cbf": cbf}

    in_maps = []
    for c in range(NCORES):
        b, chunk = c // CH, c % CH
        q0 = chunk * CS
        lo = q0 - WIN
        xh = np.zeros((HT, D), f32)
        src_lo = max(0, lo)
        xh[src_lo - lo:] = x[b, src_lo:q0 + CS]
        xh = xh.astype(bf16)
        pos_k = np.arange(lo, q0 + CS)
        cosk_a, sink_a = rope_tables(pos_k)
        # transposed multiplicative mask [r, qt, j, c]:
        #   key j_g = lo + (qt+j)*128 + r ; query i = q0 + qt*128 + c
        r_i = np.arange(128)[:, None, None, None]
        qt_i = np.arange(NQT)[None, :, None, None]
        j_i = np.arange(3)[None, None, :, None]
        c_i = np.arange(128)[None, None, None, :]
        jg = lo + (qt_i + j_i) * 128 + r_i
        gi = q0 + qt_i * 128 + c_i
        valid = (jg <= gi) & (gi - jg <= WIN) & (jg >= 0)
        maskT = valid.astype(f32).reshape(128, NQT * 3 * 128).astype(bf16)
        cbb = np.concatenate(
            [cosk_a.astype(bf16), sink_a.astype(bf16), maskT, pshuf], axis=1)
        in_maps.append(dict(common, xh=xh, cbb=np.ascontiguousarray(cbb)))
    return in_maps


_PROG = {}


def kernel(**inputs):
    from concourse.bass_utils import run_bass_kernel_spmd

    inputs = {k: np.asarray(v, dtype=np.float32) for k, v in inputs.items()}
    in_maps = prep_inputs(**inputs)
    flags = (bool(np.any(inputs["b1"] @ inputs["w_qkv"][:, 2048:])),
             bool(np.any(inputs["b_gate"] + inputs["b2"] @ inputs["w_gate"])),
             bool(np.any(inputs["b_down"])))
    if flags not in _PROG:
        _PROG[flags] = build_program(has_bv=flags[0], has_bg=flags[1],
                                     has_bd=flags[2])
    nc = _PROG[flags]
    res = run_bass_kernel_spmd(nc, in_maps, core_ids=list(range(NCORES)))
    out = np.zeros((B, S, D), np.float32)
    for c in range(NCORES):
        b, chunk = c // CH, c % CH
        out[b, chunk * CS:(chunk + 1) * CS] = res.results[c]["out"]
    return out
